# revision 22
# baseline (speedup 1.0000x reference)
"""DeepseekV3 decoder layer on 8 Trainium2 NeuronCores (Bass/Tile), v2.

Sharding: token-sharded low-rank path with LOCAL latents (no latent
AllGather): each core computes q/k/v for ALL 16 heads on its own 256
tokens, then small AllToAlls redistribute tokens->heads for attention
(2 heads/core over all tokens).  k_pe (MQA, shared) uses a tiny
AllGather.  Attention uses transposed-score layout, head-interleaved
kt loop, exp-sum accumulated on the Vector engine (one f32r matmul per
(qc,head) for the denominator), and fast approximate reciprocals.
o_proj stays output-feature-sharded fed by a per-q-chunk AllGather,
with post-LN stats via tiny AllReduce, AllGather of the normed MLP
input, and a merged FF-sharded MLP pipeline (gate/up/down + chunked
ReduceScatter per token-column block) with small tail chunks.

RMS/ln weights and the rope de-interleave are folded into the weight
matrices host-side.  RMS normalization scales are folded into the
projection OUTPUTS (projections run on raw x), removing the serial
stats->matmul dependency at stage starts.
"""

import numpy as np

B, S, H = 1, 2048, 2048
NH, NOPE, ROPE, VHD = 16, 128, 64, 128
QHD = NOPE + ROPE
QLR, KVLR, FF = 1536, 512, 8192
SCALE = QHD ** -0.5
EPS = 1e-6
NC = 8
SS = S // NC            # 256: token shard
FFS = FF // NC          # 1024: FF shard
P = 128

TRACE = False           # test.py sets kernel.TRACE = True for profiling
DEBUG = False

_CACHE = {}


def _tile_w(w):
    """[K, M] -> [K/128, ceil(M/128), 128, 128] contiguous blocks (zero-pad M)."""
    K, M = w.shape
    mc = -(-M // P)
    out = np.zeros((K // P, mc, P, P), np.float32)
    wp = np.zeros((K, mc * P), np.float32)
    wp[:, :M] = w
    for kt in range(K // P):
        for m in range(mc):
            out[kt, m] = wp[kt * P:(kt + 1) * P, m * P:(m + 1) * P]
    return out


def _build():
    if "nc" in _CACHE:
        return _CACHE["nc"]
    import concourse.mybir as mybir
    import concourse.tile as tile
    from concourse import bacc
    from concourse.masks import make_identity

    F32 = mybir.dt.float32
    F32R = mybir.dt.float32r
    BF16 = mybir.dt.bfloat16
    AF = mybir.ActivationFunctionType

    nc = bacc.Bacc("TRN2", target_bir_lowering=False, debug=False, num_devices=NC)

    def inp(name, shape, dt=F32):
        return nc.dram_tensor(name, list(shape), dt, kind="ExternalInput").ap()

    hT_s = inp("hT_s", [H, SS])               # own tokens, all features
    hT_r = inp("hT_r", [SS, S])               # own features, all tokens (resid)
    wq_a_t = inp("wq_a_t", [16, 12, P, P], BF16)
    wkv_a_t = inp("wkv_a_t", [16, 5, P, P], BF16)
    wq_b_t = inp("wq_b_t", [12, 3, P, P], BF16)    # own 2 heads
    wkv_b_t = inp("wkv_b_t", [4, 4, P, P], BF16)   # own 2 heads
    wo_t = inp("wo_t", [16, 2, P, P], BF16)
    wg_t = inp("wg_t", [16, 8, P, P], BF16)
    wu_t = inp("wu_t", [16, 8, P, P], BF16)
    wd_t = inp("wd_t", [8, 16, P, P], BF16)
    cossin = inp("cossin", [2 * P, S], BF16)        # rows 0:128 [cosT;cosT], 128:256 [sinT;sinT]
    cs_sh = inp("cs_sh", [P, SS])             # rows 0:64 cosT, 64:128 signed sinT (own shard)
    dmask = inp("dmask", [P, 4, 512], BF16)
    outT = nc.dram_tensor("outT", [SS, S], F32, kind="ExternalOutput").ap()

    RG = [list(range(NC))]
    dbg = {}
    if DEBUG:
        for nm, shp, dt in [("dbg_un", [P, 12, SS], BF16),
                            ("dbg_ckn", [P, 4, SS], BF16),
                            ("dbg_kT", [P, 2, S], BF16),
                            ("dbg_qT", [P, 2, S], BF16),
                            ("dbg_oT", [P, 2, S], BF16),
                            ("dbg_h2", [P, 2, S], F32)]:
            dbg[nm] = nc.dram_tensor(nm, shp, dt, kind="ExternalOutput").ap()

    from contextlib import ExitStack
    with tile.TileContext(nc) as tc, ExitStack() as _stack:
        cpool = _stack.enter_context(tc.tile_pool(name="const", bufs=1))
        dpool = _stack.enter_context(tc.tile_pool(name="dram", bufs=1, space="DRAM"))
        ppool = _stack.enter_context(tc.tile_pool(name="persist", bufs=1))

        # ---- DRAM collective buffers ----
        ag1a_in = dpool.tile([P, 5 * SS], BF16)
        ag1a_out = dpool.tile([NC * P, 5 * SS], BF16, addr_space="Shared")
        ag1b_in = dpool.tile([P, 12 * SS], BF16)
        ag1b_out = dpool.tile([NC * P, 12 * SS], BF16, addr_space="Shared")
        ag2_in = [dpool.tile([2 * VHD, 512], BF16, name=f"ag2_in{j}")
                  for j in range(4)]
        ag2_out = [dpool.tile([NH * VHD, 512], BF16, addr_space="Shared",
                              name=f"ag2_out{j}") for j in range(4)]
        ar4_in = [dpool.tile([1, 512], F32, name=f"ar4_in{j}") for j in range(4)]
        ar4_out = [dpool.tile([1, 512], F32, addr_space="Shared",
                              name=f"ar4_out{j}") for j in range(4)]
        ag3_in = [dpool.tile([SS, 512], BF16, name=f"ag3_in{j}") for j in range(4)]
        ag3_out = [dpool.tile([H, 512], BF16, addr_space="Shared",
                              name=f"ag3_out{j}") for j in range(4)]
        _rs_w = [512, 512, 512, 256, 128, 128]
        rs_in = [dpool.tile([H, _rs_w[j]], BF16, name=f"rs_in{j}") for j in range(6)]
        rs_out = [dpool.tile([SS, _rs_w[j]], BF16, name=f"rs_out{j}")
                  for j in range(6)]

        # ---- constants ----
        ones_f = cpool.tile([P, 1], F32)
        nc.vector.memset(ones_f[:], 1.0)
        ones_r = cpool.tile([P, 1], BF16)
        nc.vector.tensor_copy(ones_r[:], ones_f[:])
        ident_f = cpool.tile([P, P], F32)
        make_identity(nc, ident_f)
        ident_r = cpool.tile([P, P], BF16)
        nc.vector.tensor_copy(ident_r[:], ident_f[:])
        eps_t = cpool.tile([P, 1], F32)
        nc.vector.memset(eps_t[:], EPS)
        ones_k1f = cpool.tile([1, P], F32)
        nc.vector.memset(ones_k1f[:], 1.0)
        ones_k1r = cpool.tile([1, P], F32R)
        nc.vector.tensor_copy(ones_k1r[:], ones_k1f[:])
        ones_pr = cpool.tile([P, 1], F32R)
        nc.vector.tensor_copy(ones_pr[:], ones_f[:])

        def r32(ap):
            return ap.bitcast(F32R)

        h2 = ppool.tile([P, 2, S], F32)
        # carried from stage A into qkv
        un = ppool.tile([P, 12, SS], BF16)
        ckn = ppool.tile([P, 4, SS], BF16)

        # gate/up weights: loaded during qkv/attention (gpsimd-queue DMAs)
        wpool = _stack.enter_context(tc.tile_pool(name="mlpw", bufs=1))
        wgs = wpool.tile([P, 16, 8, P], BF16)
        wus = wpool.tile([P, 16, 8, P], BF16)

        # ================= Stage A: local low-rank path =================
        with tc.tile_pool(name="sa", bufs=1) as sa, \
             tc.tile_pool(name="saw", bufs=4) as saw, \
             tc.tile_pool(name="sas", bufs=3) as sas, \
             tc.tile_pool(name="pa", bufs=2, space="PSUM") as pa:
            with nc.named_scope("stageA"):
                xs = sa.tile([P, 16, SS], F32)
                nc.sync.dma_start(xs[:], hT_s.rearrange("(kt p) s -> p kt s", p=P))
                xb = sa.tile([P, 16, SS], BF16)
                for c in range(4):
                    nc.vector.tensor_copy(xb[:, 4 * c:4 * c + 4],
                                          xs[:, 4 * c:4 * c + 4])

                # all stage-A weight DMAs first (scalar queue, ahead of ACT ops)
                kvw = []
                qw = []
                for mc in range(5):
                    wt = saw.tile([P, 16, P], BF16, tag="kva", bufs=5)
                    nc.scalar.dma_start(wt[:], wkv_a_t[:, mc].rearrange("a p m -> p a m"))
                    kvw.append(wt)
                for mc in range(12):
                    wt = saw.tile([P, 16, P], BF16, tag="aw", bufs=6)
                    nc.scalar.dma_start(wt[:], wq_a_t[:, mc].rearrange("a p m -> p a m"))
                    qw.append(wt)

                # x stats (chunked squares feed accumulating ones-matmuls)
                msq1 = pa.tile([1, SS], F32, tag="msq", name="msq1")
                for c in range(4):
                    sqc = sas.tile([P, 4, SS], BF16, tag="sq")
                    nc.vector.tensor_mul(sqc[:], xs[:, 4 * c:4 * c + 4],
                                         xs[:, 4 * c:4 * c + 4])
                    for k in range(4):
                        nc.tensor.matmul(msq1[:], ones_r[:], sqc[:, k],
                                         start=(c == 0 and k == 0),
                                         stop=(c == 3 and k == 3))

                # raw kv projections first (short path -> AG1a early)
                cvs = sa.tile([P, 5, SS], F32)
                for mc in range(5):
                    ps = pa.tile([P, SS], F32, tag="amm")
                    for kt in range(16):
                        nc.tensor.matmul(ps[:], kvw[mc][:, kt], xb[:, kt],
                                         start=(kt == 0), stop=(kt == 15))
                    nc.vector.tensor_copy(cvs[:, mc], ps[:])

                r1s = sa.tile([1, SS], F32)
                nc.scalar.activation(r1s[:], msq1[:], AF.Sqrt, scale=1.0 / H,
                                     bias=eps_t[:1])
                r1sr = sa.tile([1, SS], F32R)
                nc.vector.tensor_copy(r1sr[:], r1s[:])
                r1bp = pa.tile([P, SS], F32, tag="rb", name="r1bp")
                nc.tensor.matmul(r1bp[:], ones_k1r[:], r1sr[:],
                                 start=True, stop=True)
                r1b = sa.tile([P, SS], F32)
                nc.vector.reciprocal_approx_fast(r1b[:], r1bp[:])
                r1sq = sa.tile([P, SS], F32)
                nc.vector.tensor_mul(r1sq[:], r1b[:], r1b[:])

                # kv latent stats on raw cvs; fold r1 into the scale
                sq3 = sas.tile([P, 4, SS], BF16, tag="sq")
                nc.vector.tensor_mul(sq3[:], cvs[:, :4], cvs[:, :4])
                msq3 = pa.tile([1, SS], F32, tag="msq", name="msq3")
                for k in range(4):
                    nc.tensor.matmul(msq3[:], ones_r[:], sq3[:, k],
                                     start=(k == 0), stop=(k == 3))
                msq3s = sa.tile([1, SS], F32R)
                nc.vector.tensor_copy(msq3s[:], msq3[:])
                m3bp = pa.tile([P, SS], F32, tag="rb", name="m3bp")
                nc.tensor.matmul(m3bp[:], ones_k1r[:], msq3s[:],
                                 start=True, stop=True)
                m3s = sa.tile([P, SS], F32)
                nc.vector.tensor_mul(m3s[:], m3bp[:], r1sq[:])
                r3s = sa.tile([P, SS], F32)
                nc.scalar.activation(r3s[:], m3s[:], AF.Sqrt, scale=1.0 / KVLR,
                                     bias=eps_t[:])
                r3b = sa.tile([P, SS], F32)
                nc.vector.reciprocal_approx_fast(r3b[:], r3s[:])
                s3 = sa.tile([P, SS], F32)
                nc.vector.tensor_mul(s3[:], r1b[:], r3b[:])
                nc.vector.tensor_mul(ckn[:], cvs[:, :4],
                                     s3[:, None, :].to_broadcast([P, 4, SS]))

                # k_pe rope on raw cvs[:, 4], scaled by r1
                cos_sh = sa.tile([64, SS], F32)
                nc.sync.dma_start(cos_sh[:], cs_sh[0:64, :])
                sin_sh = sa.tile([64, SS], F32)
                nc.sync.dma_start(sin_sh[:], cs_sh[64:128, :])
                ksw = sa.tile([64, SS], F32)
                nc.sync.dma_start(ksw[0:32, :], cvs[32:64, 4])
                nc.sync.dma_start(ksw[32:64, :], cvs[0:32, 4])
                kp1 = sa.tile([64, SS], F32)
                nc.vector.tensor_mul(kp1[:], cvs[:64, 4], cos_sh[:])
                kp2 = sa.tile([64, SS], F32)
                nc.vector.tensor_mul(kp2[:], ksw[:], sin_sh[:])
                nc.vector.tensor_add(kp1[:], kp1[:], kp2[:])
                kpe_n = sa.tile([64, SS], BF16)
                nc.vector.tensor_mul(kpe_n[:], kp1[:], r1b[:64, :])
                nc.sync.dma_start(
                    ag1a_in[:, 0:4 * SS].rearrange("p (kt s) -> p kt s", s=SS),
                    ckn[:])
                nc.sync.dma_start(ag1a_in[:64, 4 * SS:5 * SS], kpe_n[:])
                nc.gpsimd.collective_compute(
                    "AllGather", mybir.AluOpType.bypass, replica_groups=RG,
                    ins=[ag1a_in], outs=[ag1a_out])

                # raw q projections
                us = sa.tile([P, 12, SS], F32)
                msq2 = pa.tile([1, SS], F32, tag="msq", name="msq2")
                for mc in range(12):
                    ps = pa.tile([P, SS], F32, tag="amm")
                    for kt in range(16):
                        nc.tensor.matmul(ps[:], qw[mc][:, kt], xb[:, kt],
                                         start=(kt == 0), stop=(kt == 15))
                    nc.vector.tensor_copy(us[:, mc], ps[:])
                    sq2 = sas.tile([P, SS], BF16, tag="sq2")
                    nc.vector.tensor_mul(sq2[:], us[:, mc], us[:, mc])
                    nc.tensor.matmul(msq2[:], ones_r[:], sq2[:],
                                     start=(mc == 0), stop=(mc == 11))
                msq2s = sa.tile([1, SS], F32R)
                nc.vector.tensor_copy(msq2s[:], msq2[:])
                m2bp = pa.tile([P, SS], F32, tag="rb", name="m2bp")
                nc.tensor.matmul(m2bp[:], ones_k1r[:], msq2s[:],
                                 start=True, stop=True)
                m2s = sa.tile([P, SS], F32)
                nc.vector.tensor_mul(m2s[:], m2bp[:], r1sq[:])
                r2s = sa.tile([P, SS], F32)
                nc.scalar.activation(r2s[:], m2s[:], AF.Sqrt, scale=1.0 / QLR,
                                     bias=eps_t[:])
                r2b = sa.tile([P, SS], F32)
                nc.vector.reciprocal_approx_fast(r2b[:], r2s[:])
                s2 = sa.tile([P, SS], F32)
                nc.vector.tensor_mul(s2[:], r1b[:], r2b[:])
                nc.vector.tensor_mul(un[:], us[:],
                                     s2[:, None, :].to_broadcast([P, 12, SS]))
                nc.sync.dma_start(
                    ag1b_in.rearrange("p (kt s) -> p kt s", s=SS), un[:])
                nc.gpsimd.collective_compute(
                    "AllGather", mybir.AluOpType.bypass, replica_groups=RG,
                    ins=[ag1b_in], outs=[ag1b_out])
                for m in range(8):
                    nc.gpsimd.dma_start(wgs[:, :, m, :],
                                        wg_t[:, m].rearrange("a p m -> p a m"))
                    nc.gpsimd.dma_start(wus[:, :, m, :],
                                        wu_t[:, m].rearrange("a p m -> p a m"))
                if DEBUG:
                    nc.sync.dma_start(dbg["dbg_un"][:, :, :], un[:])
                    nc.sync.dma_start(dbg["dbg_ckn"][:, :, :], ckn[:])

        # attention-phase tiles (freed before stage D)
        bpool_cm = tc.tile_pool(name="battn", bufs=1)
        bpool = bpool_cm.__enter__()
        kvasm = bpool.tile([P, 4, S], BF16)    # [kT_h0 | kT_h1 | v_h0 | v_h1]
        v_tok = bpool.tile([P, 2, 16, P], BF16)
        qasm = bpool.tile([P, 3, S], BF16)     # [qT_h0 | qT_h1 | qpe_raw]
        qpe2 = bpool.tile([64, 2, S], BF16)
        kpeT = bpool.tile([64, S], BF16)
        oT = bpool.tile([P, 2, S], BF16)

        # ============ Stage Bq: per-2-head q/k/v from gathered latents ============
        with tc.tile_pool(name="sbw", bufs=1) as sbw, \
             tc.tile_pool(name="sbr", bufs=2) as sbr, \
             tc.tile_pool(name="sbt", bufs=2) as sbt, \
             tc.tile_pool(name="pbs", bufs=2, space="PSUM") as pbs:
            with nc.named_scope("stageB_qkv"):
                cos_t = sbw.tile([P, S], BF16)
                nc.sync.dma_start(cos_t[:], cossin[0:P, :])
                sin_t = sbw.tile([P, S], BF16)
                nc.sync.dma_start(sin_t[:], cossin[P:2 * P, :])
                wqb = sbw.tile([P, 12, 3, P], BF16)
                nc.scalar.dma_start(wqb[:], wq_b_t.rearrange("a b p m -> p a b m"))
                wkb = sbw.tile([P, 4, 4, P], BF16)
                nc.scalar.dma_start(wkb[:], wkv_b_t.rearrange("a b p m -> p a b m"))

                # kv blocks first (AG1a lands early) + V transposes
                for blk in range(8):
                    sl = slice(blk * SS, (blk + 1) * SS)
                    rhs_c = sbr.tile([P, 4, SS], BF16, tag="rhs1c")
                    nc.sync.dma_start(
                        rhs_c[:],
                        ag1a_out[blk * P:(blk + 1) * P, 0:4 * SS].rearrange(
                            "p (kt s) -> p kt s", s=SS))
                    for mc in range(4):
                        ps = pbs.tile([P, SS], F32, tag="qb")
                        for kt in range(4):
                            nc.tensor.matmul(ps[:], wkb[:, kt, mc], rhs_c[:, kt],
                                             start=(kt == 0), stop=(kt == 3))
                        nc.vector.tensor_copy(kvasm[:, mc, sl], ps[:])
                    nc.sync.dma_start(
                        kpeT[:, sl],
                        ag1a_out[blk * P:blk * P + 64, 4 * SS:5 * SS])
                    for h in range(2):
                        for st in (2 * blk, 2 * blk + 1):
                            pt = pbs.tile([P, P], BF16, tag="vtr")
                            nc.tensor.transpose(
                                pt[:], kvasm[:, 2 + h, st * P:(st + 1) * P],
                                ident_r[:])
                            nc.vector.tensor_copy(v_tok[:, h, st], pt[:])
                # q blocks (after AG1b), rope applied inline
                for blk in range(8):
                    sl = slice(blk * SS, (blk + 1) * SS)
                    rhs_u = sbr.tile([P, 12, SS], BF16, tag="rhs1")
                    nc.sync.dma_start(
                        rhs_u[:],
                        ag1b_out[blk * P:(blk + 1) * P, :].rearrange(
                            "p (kt s) -> p kt s", s=SS))
                    qpe_b = None
                    for mc in range(3):
                        ps = pbs.tile([P, SS], F32, tag="qb")
                        for kt in range(12):
                            nc.tensor.matmul(ps[:], wqb[:, kt, mc], rhs_u[:, kt],
                                             start=(kt == 0), stop=(kt == 11))
                        if mc < 2:
                            nc.vector.tensor_copy(qasm[:, mc, sl], ps[:])
                        else:
                            qpe_b = sbt.tile([P, SS], F32, tag="qpe")
                            nc.vector.tensor_copy(qpe_b[:], ps[:])
                    qsw = sbt.tile([P, SS], F32, tag="qsw")
                    for bb, sb in [(0, 1), (1, 0), (2, 3), (3, 2)]:
                        nc.sync.dma_start(qsw[32 * bb:32 * bb + 32, :],
                                          qpe_b[32 * sb:32 * sb + 32, :])
                    t1 = sbt.tile([P, SS], F32, tag="t1")
                    nc.vector.tensor_mul(t1[:], qpe_b[:], cos_t[:, sl])
                    t2 = sbt.tile([P, SS], F32, tag="t2")
                    nc.vector.tensor_mul(t2[:], qsw[:], sin_t[:, sl])
                    qrot = sbt.tile([P, SS], BF16, tag="qrot")
                    nc.vector.tensor_add(qrot[:], t1[:], t2[:])
                    nc.scalar.dma_start(qpe2[:, 0, sl], qrot[0:64, :])
                    nc.scalar.dma_start(qpe2[:, 1, sl], qrot[64:128, :])

                if DEBUG:
                    nc.sync.dma_start(dbg["dbg_kT"][:, :, :], kvasm[:, 0:2])
                    nc.sync.dma_start(dbg["dbg_qT"][:, :, :], qasm[:, 0:2])

        # ================= attention + pipelined o_proj/post-LN =================
        with tc.tile_pool(name="sbe", bufs=6) as sbe, \
             tc.tile_pool(name="sbm", bufs=1) as sbm, \
             tc.tile_pool(name="scs", bufs=1) as scs, \
             tc.tile_pool(name="scr", bufs=1) as scr, \
             tc.tile_pool(name="pat", bufs=1, space="PSUM") as pat:
            wos = sbm.tile([P, 16, 2, P], BF16)
            nc.sync.dma_start(wos[:], wo_t.rearrange("a b p m -> p a b m"))

            def cproj_a(j):
                """o_proj chunk j + residual + sq-stats + AR4_j."""
                nsl = slice(j * 512, (j + 1) * 512)
                rhs = scr.tile([P, 16, 512], BF16, tag="rhs2", name="rhs2")
                nc.sync.dma_start(
                    rhs[:], ag2_out[j].rearrange("(kt p) s -> p kt s", p=P))
                resid = scs.tile([P, 2, 512], F32, tag="resid", name="resid")
                nc.sync.dma_start(
                    resid[:],
                    hT_r.rearrange("(mc p) s -> p mc s", p=P)[:, :, nsl])
                sqh = scs.tile([P, 2, 512], BF16, tag="sqh", name="sqh")
                for mc in range(2):
                    ps = pat.tile([P, 512], F32, tag="rb", bufs=1, name="omm")
                    for kt in range(16):
                        nc.tensor.matmul(ps[:], wos[:, kt, mc], rhs[:, kt],
                                         start=(kt == 0), stop=(kt == 15))
                    nc.vector.tensor_add(h2[:, mc, nsl], ps[:], resid[:, mc])
                    nc.vector.tensor_mul(sqh[:, mc], h2[:, mc, nsl],
                                         h2[:, mc, nsl])
                ps4 = pat.tile([1, 512], F32, tag="m4", bufs=1, name="m4")
                for mc in range(2):
                    nc.tensor.matmul(ps4[:], ones_r[:], sqh[:, mc],
                                     start=(mc == 0), stop=(mc == 1))
                msq4 = scs.tile([1, 512], F32, tag="msq4", name="msq4")
                nc.vector.tensor_copy(msq4[:], ps4[:])
                nc.sync.dma_start(ar4_in[j][:, :], msq4[:])
                nc.gpsimd.collective_compute(
                    "AllReduce", mybir.AluOpType.add, replica_groups=RG,
                    ins=[ar4_in[j]], outs=[ar4_out[j]])

            def cproj_b(j):
                """r4_j + yT_j + AG3_j."""
                nsl = slice(j * 512, (j + 1) * 512)
                msq4g = scs.tile([1, 512], F32, tag="m4g", name="msq4g")
                nc.sync.dma_start(msq4g[:], ar4_out[j][:, :])
                r4s = scs.tile([1, 512], F32, tag="r4s", name="r4s")
                nc.scalar.activation(r4s[:], msq4g[:], AF.Sqrt,
                                     scale=1.0 / H, bias=eps_t[:1])
                r4sr = scs.tile([1, 512], F32R, tag="r4sr", name="r4sr")
                nc.vector.tensor_copy(r4sr[:], r4s[:])
                r4bp = pat.tile([P, 512], F32, tag="rb", bufs=1, name="rb")
                nc.tensor.matmul(r4bp[:], ones_k1r[:], r4sr[:],
                                 start=True, stop=True)
                r4b = scs.tile([P, 512], F32, tag="r4b", name="r4b")
                nc.vector.reciprocal_approx_fast(r4b[:], r4bp[:])
                yT = scs.tile([P, 2, 512], BF16, tag="yT", name="yT")
                nc.vector.tensor_mul(
                    yT[:], h2[:, :, nsl],
                    r4b[:, None, :].to_broadcast([P, 2, 512]))
                nc.sync.dma_start(
                    ag3_in[j].rearrange("(mc p) s -> p mc s", p=P), yT[:])
                nc.gpsimd.collective_compute(
                    "AllGather", mybir.AluOpType.bypass, replica_groups=RG,
                    ins=[ag3_in[j]], outs=[ag3_out[j]])

            with nc.named_scope("stageB_attn"):
                mask_t = sbm.tile([P, 4, 512], BF16)
                nc.sync.dma_start(mask_t[:], dmask[:, :, :])
                for qc in range(4):
                    qsl = slice(qc * 512, (qc + 1) * 512)
                    nkt = 4 * qc + 4
                    o_ps = [pat.tile([P, 512], F32, tag="o", bufs=2,
                                     name=f"o{h}") for h in range(2)]
                    d_ps = [pat.tile([1, 512], F32, tag="d", bufs=2,
                                     name=f"d{h}") for h in range(2)]
                    for kt in range(nkt):
                        ksl = slice(kt * P, (kt + 1) * P)
                        j = kt - 4 * qc
                        for h in range(2):
                            sc_ps = pat.tile([P, 512], F32, tag="sc", bufs=2,
                                             name="scp")
                            nc.tensor.matmul(sc_ps[:], kvasm[:, h, ksl],
                                             qasm[:, h, qsl], start=True,
                                             stop=False)
                            nc.tensor.matmul(sc_ps[:], kpeT[:, ksl],
                                             qpe2[:, h, qsl], start=False,
                                             stop=True)
                            if j >= 0:
                                nc.vector.tensor_add(sc_ps[:], sc_ps[:],
                                                     mask_t[:, j])
                            es = sbe.tile([P, 512], BF16, tag="es", bufs=4)
                            nc.scalar.activation(es[:], sc_ps[:], AF.Exp)
                            nc.tensor.matmul(o_ps[h][:], v_tok[:, h, kt], es[:],
                                             start=(kt == 0), stop=(kt == nkt - 1))
                            nc.tensor.matmul(d_ps[h][:], ones_r[:], es[:],
                                             start=(kt == 0), stop=(kt == nkt - 1))
                    for h in range(2):
                        ds = sbe.tile([1, 512], F32R, tag="ds", bufs=2)
                        nc.vector.tensor_copy(ds[:], d_ps[h][:])
                        rb_ps = pat.tile([P, 512], F32, tag="rb", bufs=1,
                                         name="rbo")
                        nc.tensor.matmul(rb_ps[:], ones_k1r[:], ds[:],
                                         start=True, stop=True)
                        recb = sbe.tile([P, 512], F32, tag="recb", bufs=2)
                        nc.vector.reciprocal_approx_fast(recb[:], rb_ps[:])
                        nc.vector.tensor_mul(oT[:, h, qsl], o_ps[h][:], recb[:])
                    nc.sync.dma_start(
                        ag2_in[qc].rearrange("(mc p) s -> p mc s", p=P),
                        oT[:, :, qsl])
                    nc.gpsimd.collective_compute(
                        "AllGather", mybir.AluOpType.bypass, replica_groups=RG,
                        ins=[ag2_in[qc]], outs=[ag2_out[qc]])
                    if qc >= 1:
                        cproj_a(qc - 1)
                    if qc >= 2:
                        cproj_b(qc - 2)
                cproj_a(3)
                cproj_b(2)
                cproj_b(3)
                if DEBUG:
                    nc.sync.dma_start(dbg["dbg_oT"][:, :, :], oT[:])
                    nc.sync.dma_start(dbg["dbg_h2"][:, :, :], h2[:])

        bpool_cm.__exit__(None, None, None)

        # ================= Stage D: merged MLP pipeline =================
        with tc.tile_pool(name="sd", bufs=1) as sd, \
             tc.tile_pool(name="sdr", bufs=2) as sdr, \
             tc.tile_pool(name="sde", bufs=4) as sde, \
             tc.tile_pool(name="sdd", bufs=6) as sdd, \
             tc.tile_pool(name="pdg", bufs=2, space="PSUM") as pdg:
            with nc.named_scope("stageD"):
                wds = sd.tile([P, 8, 16, P], BF16)
                for mc in range(16):
                    nc.gpsimd.dma_start(wds[:, :, mc, :],
                                        wd_t[:, mc].rearrange("a p m -> p a m"))
                act = sd.tile([P, 8, 512], BF16, name="act")  # per-ncol activations

                # rs chunk layout: (ncol, col offset within ncol, width, rs idx)
                CH = {0: [(0, 512, 0)], 1: [(0, 512, 1)], 2: [(0, 512, 2)],
                      3: [(0, 256, 3), (256, 128, 4), (384, 128, 5)]}
                for ncol in range(4):
                    nsl = slice(ncol * 512, (ncol + 1) * 512)
                    rhs = sdr.tile([P, 16, 512], BF16, tag="rhs3")
                    nc.sync.dma_start(
                        rhs[:], ag3_out[ncol].rearrange("(kt p) s -> p kt s", p=P))
                    for m in range(8):
                        gp = pdg.tile([P, 512], F32, tag="g", name="gps")
                        up = pdg.tile([P, 512], F32, tag="u", name="ups")
                        for kt in range(16):
                            nc.tensor.matmul(gp[:], wgs[:, kt, m], rhs[:, kt],
                                             start=(kt == 0), stop=(kt == 15))
                            nc.tensor.matmul(up[:], wus[:, kt, m], rhs[:, kt],
                                             start=(kt == 0), stop=(kt == 15))
                        gsil = sde.tile([P, 512], BF16, tag="gsil")
                        nc.scalar.activation(gsil[:], gp[:], AF.Silu)
                        nc.vector.tensor_mul(act[:, m], gsil[:], up[:])
                    # down projection for this token-column block
                    for (c0, cw, jr) in CH[ncol]:
                        csl = slice(c0, c0 + cw)
                        gsl = slice(ncol * 512 + c0, ncol * 512 + c0 + cw)
                        for mc in range(16):
                            ps = pdg.tile([P, 512], F32, tag="dmm",
                                          name="dmmps")[:, :cw]
                            for kt in range(8):
                                nc.tensor.matmul(ps[:], wds[:, kt, mc],
                                                 act[:, kt, csl],
                                                 start=(kt == 0), stop=(kt == 7))
                            dn = sdd.tile([P, 512], BF16, tag="dn",
                                          name="dntile")[:, :cw]
                            if mc % 2 == 0:
                                nc.vector.tensor_copy(dn[:], ps[:])
                            else:
                                nc.scalar.activation(dn[:], ps[:], AF.Copy)
                            nc.sync.dma_start(rs_in[jr][mc * P:(mc + 1) * P, :],
                                              dn[:])
                        nc.gpsimd.collective_compute(
                            "ReduceScatter", mybir.AluOpType.add,
                            replica_groups=RG,
                            ins=[rs_in[jr]], outs=[rs_out[jr]])
                        fin = sdd.tile([P, 2, 512], BF16, tag="fin", bufs=2,
                                       name="fintile")[:, :, :cw]
                        nc.sync.dma_start(
                            fin[:],
                            rs_out[jr].rearrange("(mc p) s -> p mc s", p=P))
                        fino = sdd.tile([P, 2, 512], F32, tag="fino", bufs=2,
                                        name="finotile")[:, :, :cw]
                        nc.vector.tensor_add(fino[:], fin[:], h2[:, :, gsl])
                        nc.sync.dma_start(
                            outT.rearrange("(mc p) s -> p mc s", p=P)[:, :, gsl],
                            fino[:])

    nc.compile()
    _CACHE["nc"] = nc
    return nc


def _host_prep(inputs):
    import ml_dtypes
    bf16 = ml_dtypes.bfloat16
    inp = {k: np.asarray(v) for k, v in inputs.items()}
    hidden = inp["hidden_states"].reshape(S, H).astype(np.float32)
    pos = inp["position_ids"].reshape(S).astype(np.int64)
    cosT = inp["cos"][pos].T.astype(np.float32)
    sinT = inp["sin"][pos].T.astype(np.float32)
    wq_a = (inp["wq_a"] * inp["in_ln"][:, None]).astype(np.float32)
    wkv_a = (inp["wkv_a"] * inp["in_ln"][:, None]).astype(np.float32)
    wq_b = (inp["wq_b"] * inp["q_a_ln"][:, None]).astype(np.float32)
    wkv_b = (inp["wkv_b"] * inp["kv_a_ln"][:, None]).astype(np.float32)
    wg = (inp["w_gate"] * inp["post_ln"][:, None]).astype(np.float32)
    wu = (inp["w_up"] * inp["post_ln"][:, None]).astype(np.float32)
    wd = inp["w_down"].astype(np.float32)
    wo = inp["wo"].astype(np.float32)

    de = np.empty(ROPE, np.int64)
    de[:32] = np.arange(32) * 2
    de[32:] = np.arange(32) * 2 + 1
    wkv_a = np.concatenate([wkv_a[:, :KVLR], wkv_a[:, KVLR:][:, de]], axis=1)
    wq_b = wq_b.reshape(QLR, NH, QHD)
    wkv_b = wkv_b.reshape(KVLR, NH, NOPE + VHD)

    hT = hidden.T.copy()
    sin_sg = np.concatenate([-sinT[:32], sinT[32:]], axis=0)    # signed for swap trick
    cossin = np.concatenate([cosT, cosT, sin_sg, sin_sg], axis=0)  # (256, S)
    ki = np.arange(P)[:, None]
    qi = np.arange(512)[None, :]
    dmask = np.stack([np.where(qi >= j * P + ki, 0.0, -1e30).astype(np.float32)
                      for j in range(4)], axis=1).astype(bf16)  # (128, 4, 512)

    wq_a_t = _tile_w(wq_a).astype(bf16)
    wkv_a_t = _tile_w(wkv_a).astype(bf16)

    in_maps = []
    for c in range(NC):
        h0, h1 = 2 * c, 2 * c + 1
        qb = np.concatenate([
            wq_b[:, h0, :NOPE], wq_b[:, h1, :NOPE],
            wq_b[:, h0, NOPE:][:, de], wq_b[:, h1, NOPE:][:, de]], axis=1) * SCALE
        kb = np.concatenate([
            wkv_b[:, h0, :NOPE], wkv_b[:, h1, :NOPE],
            wkv_b[:, h0, NOPE:], wkv_b[:, h1, NOPE:]], axis=1)
        ssl = slice(c * SS, (c + 1) * SS)
        cs_sh = np.concatenate([cosT[:, ssl], sin_sg[:, ssl]], axis=0)
        in_maps.append({
            "hT_s": np.ascontiguousarray(hT[:, ssl]),
            "hT_r": np.ascontiguousarray(hT[ssl, :]),
            "wq_a_t": wq_a_t,
            "wkv_a_t": wkv_a_t,
            "wq_b_t": _tile_w(qb.astype(np.float32)).astype(bf16),
            "wkv_b_t": _tile_w(kb.astype(np.float32)).astype(bf16),
            "wo_t": _tile_w(np.ascontiguousarray(wo[:, ssl])).astype(bf16),
            "wg_t": _tile_w(wg[:, c * FFS:(c + 1) * FFS]).astype(bf16),
            "wu_t": _tile_w(wu[:, c * FFS:(c + 1) * FFS]).astype(bf16),
            "wd_t": _tile_w(wd[c * FFS:(c + 1) * FFS, :]).astype(bf16),
            "cossin": cossin.astype(bf16),
            "cs_sh": np.ascontiguousarray(cs_sh),
            "dmask": dmask,
        })
    return in_maps


_LAST_RESULT = {}


def kernel(**inputs) -> np.ndarray:
    from concourse.bass_utils import run_bass_kernel_spmd
    nc = _build()
    in_maps = _host_prep(inputs)
    kwargs = {}
    if TRACE:
        import sys, types
        if "antenv.axon_hooks" not in sys.modules:
            try:
                from trn_agent_boot.trn_boot import _ntff_profile_via_ctypes
                mod = types.ModuleType("antenv.axon_hooks")
                _hook = _ntff_profile_via_ctypes('/opt/axon/libaxon_pjrt.so')
                mod.get_axon_ntff_profile_hook = lambda: _hook
                mod.set_axon_ntff_profile_hook = lambda h: None
                sys.modules["antenv.axon_hooks"] = mod
                import antenv
                antenv.axon_hooks = mod
            except Exception:
                pass
        kwargs["trace"] = True
    res = run_bass_kernel_spmd(nc, in_maps, list(range(NC)), **kwargs)
    _LAST_RESULT["res"] = res
    outT = np.concatenate([res.results[c]["outT"] for c in range(NC)], axis=0)
    return np.ascontiguousarray(outT.T)[None].astype(np.float32)


# revision 24
# speedup vs baseline: 1.0139x; 1.0139x over previous
"""DeepseekV3 decoder layer on 8 Trainium2 NeuronCores (Bass/Tile), v2.

Sharding: token-sharded low-rank path with LOCAL latents (no latent
AllGather): each core computes q/k/v for ALL 16 heads on its own 256
tokens, then small AllToAlls redistribute tokens->heads for attention
(2 heads/core over all tokens).  k_pe (MQA, shared) uses a tiny
AllGather.  Attention uses transposed-score layout, head-interleaved
kt loop, exp-sum accumulated on the Vector engine (one f32r matmul per
(qc,head) for the denominator), and fast approximate reciprocals.
o_proj stays output-feature-sharded fed by a per-q-chunk AllGather,
with post-LN stats via tiny AllReduce, AllGather of the normed MLP
input, and a merged FF-sharded MLP pipeline (gate/up/down + chunked
ReduceScatter per token-column block) with small tail chunks.

RMS/ln weights and the rope de-interleave are folded into the weight
matrices host-side.  RMS normalization scales are folded into the
projection OUTPUTS (projections run on raw x), removing the serial
stats->matmul dependency at stage starts.
"""

import numpy as np

B, S, H = 1, 2048, 2048
NH, NOPE, ROPE, VHD = 16, 128, 64, 128
QHD = NOPE + ROPE
QLR, KVLR, FF = 1536, 512, 8192
SCALE = QHD ** -0.5
EPS = 1e-6
NC = 8
SS = S // NC            # 256: token shard
FFS = FF // NC          # 1024: FF shard
P = 128

TRACE = False           # test.py sets kernel.TRACE = True for profiling
DEBUG = False

_CACHE = {}


def _tile_w(w):
    """[K, M] -> [K/128, ceil(M/128), 128, 128] contiguous blocks (zero-pad M)."""
    K, M = w.shape
    mc = -(-M // P)
    out = np.zeros((K // P, mc, P, P), np.float32)
    wp = np.zeros((K, mc * P), np.float32)
    wp[:, :M] = w
    for kt in range(K // P):
        for m in range(mc):
            out[kt, m] = wp[kt * P:(kt + 1) * P, m * P:(m + 1) * P]
    return out


def _build():
    if "nc" in _CACHE:
        return _CACHE["nc"]
    import concourse.mybir as mybir
    import concourse.tile as tile
    from concourse import bacc
    from concourse.masks import make_identity

    F32 = mybir.dt.float32
    F32R = mybir.dt.float32r
    BF16 = mybir.dt.bfloat16
    AF = mybir.ActivationFunctionType

    nc = bacc.Bacc("TRN2", target_bir_lowering=False, debug=False, num_devices=NC)

    def inp(name, shape, dt=F32):
        return nc.dram_tensor(name, list(shape), dt, kind="ExternalInput").ap()

    hT_s = inp("hT_s", [H, SS])               # own tokens, all features
    hT_r = inp("hT_r", [SS, S])               # own features, all tokens (resid)
    wq_a_t = inp("wq_a_t", [16, 12, P, P], BF16)
    wkv_a_t = inp("wkv_a_t", [16, 5, P, P], BF16)
    wq_b_t = inp("wq_b_t", [12, 3, P, P], BF16)    # own 2 heads
    wkv_b_t = inp("wkv_b_t", [4, 4, P, P], BF16)   # own 2 heads
    wo_t = inp("wo_t", [16, 2, P, P], BF16)
    wg_t = inp("wg_t", [16, 8, P, P], BF16)
    wu_t = inp("wu_t", [16, 8, P, P], BF16)
    wd_t = inp("wd_t", [8, 16, P, P], BF16)
    cossin = inp("cossin", [2 * P, S], BF16)        # rows 0:128 [cosT;cosT], 128:256 [sinT;sinT]
    cs_sh = inp("cs_sh", [P, SS])             # rows 0:64 cosT, 64:128 signed sinT (own shard)
    dmask = inp("dmask", [P, 4, 512], BF16)
    outT = nc.dram_tensor("outT", [SS, S], F32, kind="ExternalOutput").ap()

    RG = [list(range(NC))]
    dbg = {}
    if DEBUG:
        for nm, shp, dt in [("dbg_un", [P, 12, SS], BF16),
                            ("dbg_ckn", [P, 4, SS], BF16),
                            ("dbg_kT", [P, 2, S], BF16),
                            ("dbg_qT", [P, 2, S], BF16),
                            ("dbg_oT", [P, 2, S], BF16),
                            ("dbg_h2", [P, 2, S], F32)]:
            dbg[nm] = nc.dram_tensor(nm, shp, dt, kind="ExternalOutput").ap()

    from contextlib import ExitStack
    with tile.TileContext(nc) as tc, ExitStack() as _stack:
        cpool = _stack.enter_context(tc.tile_pool(name="const", bufs=1))
        dpool = _stack.enter_context(tc.tile_pool(name="dram", bufs=1, space="DRAM"))
        ppool = _stack.enter_context(tc.tile_pool(name="persist", bufs=1))

        # ---- DRAM collective buffers ----
        ag1a_in = dpool.tile([P, 5 * SS], BF16)
        ag1a_out = dpool.tile([NC * P, 5 * SS], BF16, addr_space="Shared")
        ag1b_in = dpool.tile([P, 12 * SS], BF16)
        ag1b_out = dpool.tile([NC * P, 12 * SS], BF16, addr_space="Shared")
        ag2_in = [dpool.tile([2 * VHD, 512], BF16, name=f"ag2_in{j}")
                  for j in range(4)]
        ag2_out = [dpool.tile([NH * VHD, 512], BF16, addr_space="Shared",
                              name=f"ag2_out{j}") for j in range(4)]
        ar4_in = [dpool.tile([1, 512], F32, name=f"ar4_in{j}") for j in range(4)]
        ar4_out = [dpool.tile([1, 512], F32, addr_space="Shared",
                              name=f"ar4_out{j}") for j in range(4)]
        ag3_in = [dpool.tile([SS, 512], BF16, name=f"ag3_in{j}") for j in range(4)]
        ag3_out = [dpool.tile([H, 512], BF16, addr_space="Shared",
                              name=f"ag3_out{j}") for j in range(4)]
        _rs_w = [512, 512, 512, 256, 256]
        rs_in = [dpool.tile([H, _rs_w[j]], BF16, name=f"rs_in{j}") for j in range(5)]
        rs_out = [dpool.tile([SS, _rs_w[j]], BF16, name=f"rs_out{j}")
                  for j in range(5)]

        # ---- constants ----
        ones_f = cpool.tile([P, 1], F32)
        nc.vector.memset(ones_f[:], 1.0)
        ones_r = cpool.tile([P, 1], BF16)
        nc.vector.tensor_copy(ones_r[:], ones_f[:])
        ident_f = cpool.tile([P, P], F32)
        make_identity(nc, ident_f)
        ident_r = cpool.tile([P, P], BF16)
        nc.vector.tensor_copy(ident_r[:], ident_f[:])
        eps_t = cpool.tile([P, 1], F32)
        nc.vector.memset(eps_t[:], EPS)
        ones_k1f = cpool.tile([1, P], F32)
        nc.vector.memset(ones_k1f[:], 1.0)
        ones_k1r = cpool.tile([1, P], F32R)
        nc.vector.tensor_copy(ones_k1r[:], ones_k1f[:])
        ones_pr = cpool.tile([P, 1], F32R)
        nc.vector.tensor_copy(ones_pr[:], ones_f[:])

        def r32(ap):
            return ap.bitcast(F32R)

        h2 = ppool.tile([P, 2, S], F32)
        # carried from stage A into qkv
        un = ppool.tile([P, 12, SS], BF16)
        ckn = ppool.tile([P, 4, SS], BF16)

        # gate/up weights: loaded during qkv/attention (gpsimd-queue DMAs)
        wpool = _stack.enter_context(tc.tile_pool(name="mlpw", bufs=1))
        wgs = wpool.tile([P, 16, 8, P], BF16)
        wus = wpool.tile([P, 16, 8, P], BF16)

        # ================= Stage A: local low-rank path =================
        with tc.tile_pool(name="sa", bufs=1) as sa, \
             tc.tile_pool(name="saw", bufs=4) as saw, \
             tc.tile_pool(name="sas", bufs=3) as sas, \
             tc.tile_pool(name="pa", bufs=2, space="PSUM") as pa:
            with nc.named_scope("stageA"):
                xs = sa.tile([P, 16, SS], F32)
                nc.sync.dma_start(xs[:], hT_s.rearrange("(kt p) s -> p kt s", p=P))
                xb = sa.tile([P, 16, SS], BF16)
                for c in range(4):
                    nc.vector.tensor_copy(xb[:, 4 * c:4 * c + 4],
                                          xs[:, 4 * c:4 * c + 4])

                # all stage-A weight DMAs first (scalar queue, ahead of ACT ops)
                kvw = []
                qw = []
                for mc in range(5):
                    wt = saw.tile([P, 16, P], BF16, tag="kva", bufs=5)
                    nc.scalar.dma_start(wt[:], wkv_a_t[:, mc].rearrange("a p m -> p a m"))
                    kvw.append(wt)
                for mc in range(12):
                    wt = saw.tile([P, 16, P], BF16, tag="aw", bufs=6)
                    nc.scalar.dma_start(wt[:], wq_a_t[:, mc].rearrange("a p m -> p a m"))
                    qw.append(wt)

                # x stats (chunked squares feed accumulating ones-matmuls)
                msq1 = pa.tile([1, SS], F32, tag="msq", name="msq1")
                for c in range(4):
                    sqc = sas.tile([P, 4, SS], BF16, tag="sq")
                    nc.vector.tensor_mul(sqc[:], xs[:, 4 * c:4 * c + 4],
                                         xs[:, 4 * c:4 * c + 4])
                    for k in range(4):
                        nc.tensor.matmul(msq1[:], ones_r[:], sqc[:, k],
                                         start=(c == 0 and k == 0),
                                         stop=(c == 3 and k == 3))

                # raw kv projections first (short path -> AG1a early)
                cvs = sa.tile([P, 5, SS], F32)
                for mc in range(5):
                    ps = pa.tile([P, SS], F32, tag="amm")
                    for kt in range(16):
                        nc.tensor.matmul(ps[:], kvw[mc][:, kt], xb[:, kt],
                                         start=(kt == 0), stop=(kt == 15))
                    nc.vector.tensor_copy(cvs[:, mc], ps[:])

                r1s = sa.tile([1, SS], F32)
                nc.scalar.activation(r1s[:], msq1[:], AF.Sqrt, scale=1.0 / H,
                                     bias=eps_t[:1])
                r1sr = sa.tile([1, SS], F32R)
                nc.vector.tensor_copy(r1sr[:], r1s[:])
                r1bp = pa.tile([P, SS], F32, tag="rb", name="r1bp")
                nc.tensor.matmul(r1bp[:], ones_k1r[:], r1sr[:],
                                 start=True, stop=True)
                r1b = sa.tile([P, SS], F32)
                nc.vector.reciprocal_approx_fast(r1b[:], r1bp[:])
                r1sq = sa.tile([P, SS], F32)
                nc.vector.tensor_mul(r1sq[:], r1b[:], r1b[:])

                # kv latent stats on raw cvs; fold r1 into the scale
                sq3 = sas.tile([P, 4, SS], BF16, tag="sq")
                nc.vector.tensor_mul(sq3[:], cvs[:, :4], cvs[:, :4])
                msq3 = pa.tile([1, SS], F32, tag="msq", name="msq3")
                for k in range(4):
                    nc.tensor.matmul(msq3[:], ones_r[:], sq3[:, k],
                                     start=(k == 0), stop=(k == 3))
                msq3s = sa.tile([1, SS], F32R)
                nc.vector.tensor_copy(msq3s[:], msq3[:])
                m3bp = pa.tile([P, SS], F32, tag="rb", name="m3bp")
                nc.tensor.matmul(m3bp[:], ones_k1r[:], msq3s[:],
                                 start=True, stop=True)
                m3s = sa.tile([P, SS], F32)
                nc.vector.tensor_mul(m3s[:], m3bp[:], r1sq[:])
                r3s = sa.tile([P, SS], F32)
                nc.scalar.activation(r3s[:], m3s[:], AF.Sqrt, scale=1.0 / KVLR,
                                     bias=eps_t[:])
                r3b = sa.tile([P, SS], F32)
                nc.vector.reciprocal_approx_fast(r3b[:], r3s[:])
                s3 = sa.tile([P, SS], F32)
                nc.vector.tensor_mul(s3[:], r1b[:], r3b[:])
                nc.vector.tensor_mul(ckn[:], cvs[:, :4],
                                     s3[:, None, :].to_broadcast([P, 4, SS]))

                # k_pe rope on raw cvs[:, 4], scaled by r1
                cos_sh = sa.tile([64, SS], F32)
                nc.sync.dma_start(cos_sh[:], cs_sh[0:64, :])
                sin_sh = sa.tile([64, SS], F32)
                nc.sync.dma_start(sin_sh[:], cs_sh[64:128, :])
                ksw = sa.tile([64, SS], F32)
                nc.sync.dma_start(ksw[0:32, :], cvs[32:64, 4])
                nc.sync.dma_start(ksw[32:64, :], cvs[0:32, 4])
                kp1 = sa.tile([64, SS], F32)
                nc.vector.tensor_mul(kp1[:], cvs[:64, 4], cos_sh[:])
                kp2 = sa.tile([64, SS], F32)
                nc.vector.tensor_mul(kp2[:], ksw[:], sin_sh[:])
                nc.vector.tensor_add(kp1[:], kp1[:], kp2[:])
                kpe_n = sa.tile([64, SS], BF16)
                nc.vector.tensor_mul(kpe_n[:], kp1[:], r1b[:64, :])
                nc.sync.dma_start(
                    ag1a_in[:, 0:4 * SS].rearrange("p (kt s) -> p kt s", s=SS),
                    ckn[:])
                nc.sync.dma_start(ag1a_in[:64, 4 * SS:5 * SS], kpe_n[:])
                nc.gpsimd.collective_compute(
                    "AllGather", mybir.AluOpType.bypass, replica_groups=RG,
                    ins=[ag1a_in], outs=[ag1a_out])

                # raw q projections
                us = sa.tile([P, 12, SS], F32)
                msq2 = pa.tile([1, SS], F32, tag="msq", name="msq2")
                for mc in range(12):
                    ps = pa.tile([P, SS], F32, tag="amm")
                    for kt in range(16):
                        nc.tensor.matmul(ps[:], qw[mc][:, kt], xb[:, kt],
                                         start=(kt == 0), stop=(kt == 15))
                    nc.vector.tensor_copy(us[:, mc], ps[:])
                    sq2 = sas.tile([P, SS], BF16, tag="sq2")
                    nc.vector.tensor_mul(sq2[:], us[:, mc], us[:, mc])
                    nc.tensor.matmul(msq2[:], ones_r[:], sq2[:],
                                     start=(mc == 0), stop=(mc == 11))
                msq2s = sa.tile([1, SS], F32R)
                nc.vector.tensor_copy(msq2s[:], msq2[:])
                m2bp = pa.tile([P, SS], F32, tag="rb", name="m2bp")
                nc.tensor.matmul(m2bp[:], ones_k1r[:], msq2s[:],
                                 start=True, stop=True)
                m2s = sa.tile([P, SS], F32)
                nc.vector.tensor_mul(m2s[:], m2bp[:], r1sq[:])
                r2s = sa.tile([P, SS], F32)
                nc.scalar.activation(r2s[:], m2s[:], AF.Sqrt, scale=1.0 / QLR,
                                     bias=eps_t[:])
                r2b = sa.tile([P, SS], F32)
                nc.vector.reciprocal_approx_fast(r2b[:], r2s[:])
                s2 = sa.tile([P, SS], F32)
                nc.vector.tensor_mul(s2[:], r1b[:], r2b[:])
                nc.vector.tensor_mul(un[:], us[:],
                                     s2[:, None, :].to_broadcast([P, 12, SS]))
                nc.sync.dma_start(
                    ag1b_in.rearrange("p (kt s) -> p kt s", s=SS), un[:])
                nc.gpsimd.collective_compute(
                    "AllGather", mybir.AluOpType.bypass, replica_groups=RG,
                    ins=[ag1b_in], outs=[ag1b_out])
                for m in range(8):
                    nc.vector.tensor_copy(wgs[0:1, 0:1, m, 0:1],
                                          ckn[0:1, 0:1, 0:1])
                    nc.vector.tensor_copy(wus[0:1, 0:1, m, 0:1],
                                          ckn[0:1, 0:1, 0:1])
                    nc.gpsimd.dma_start(wgs[:, :, m, :],
                                        wg_t[:, m].rearrange("a p m -> p a m"))
                    nc.gpsimd.dma_start(wus[:, :, m, :],
                                        wu_t[:, m].rearrange("a p m -> p a m"))
                if DEBUG:
                    nc.sync.dma_start(dbg["dbg_un"][:, :, :], un[:])
                    nc.sync.dma_start(dbg["dbg_ckn"][:, :, :], ckn[:])

        # attention-phase tiles (freed before stage D)
        bpool_cm = tc.tile_pool(name="battn", bufs=1)
        bpool = bpool_cm.__enter__()
        kvasm = bpool.tile([P, 4, S], BF16)    # [kT_h0 | kT_h1 | v_h0 | v_h1]
        v_tok = bpool.tile([P, 2, 16, P], BF16)
        qasm = bpool.tile([P, 2, S], BF16)     # [qT_h0 | qT_h1]
        qpe2 = bpool.tile([64, 2, S], BF16)
        kpeT = bpool.tile([64, S], BF16)
        oT = bpool.tile([P, 2, S], BF16)

        # ============ Stage Bq: per-2-head q/k/v from gathered latents ============
        with tc.tile_pool(name="sbw", bufs=1) as sbw, \
             tc.tile_pool(name="sbr", bufs=2) as sbr, \
             tc.tile_pool(name="sbt", bufs=2) as sbt, \
             tc.tile_pool(name="pbs", bufs=2, space="PSUM") as pbs:
            with nc.named_scope("stageB_qkv"):
                cos_t = sbw.tile([P, S], BF16)
                nc.sync.dma_start(cos_t[:], cossin[0:P, :])
                sin_t = sbw.tile([P, S], BF16)
                nc.sync.dma_start(sin_t[:], cossin[P:2 * P, :])
                wqb = sbw.tile([P, 12, 3, P], BF16)
                nc.scalar.dma_start(wqb[:], wq_b_t.rearrange("a b p m -> p a b m"))
                wkb = sbw.tile([P, 4, 4, P], BF16)
                nc.scalar.dma_start(wkb[:], wkv_b_t.rearrange("a b p m -> p a b m"))

                # kv blocks first (AG1a lands early) + V transposes
                for blk in range(8):
                    sl = slice(blk * SS, (blk + 1) * SS)
                    rhs_c = sbr.tile([P, 4, SS], BF16, tag="rhs1c")
                    nc.sync.dma_start(
                        rhs_c[:],
                        ag1a_out[blk * P:(blk + 1) * P, 0:4 * SS].rearrange(
                            "p (kt s) -> p kt s", s=SS))
                    for mc in range(4):
                        ps = pbs.tile([P, SS], F32, tag="qb")
                        for kt in range(4):
                            nc.tensor.matmul(ps[:], wkb[:, kt, mc], rhs_c[:, kt],
                                             start=(kt == 0), stop=(kt == 3))
                        nc.vector.tensor_copy(kvasm[:, mc, sl], ps[:])
                    nc.sync.dma_start(
                        kpeT[:, sl],
                        ag1a_out[blk * P:blk * P + 64, 4 * SS:5 * SS])
                    for h in range(2):
                        for st in (2 * blk, 2 * blk + 1):
                            pt = pbs.tile([P, P], BF16, tag="vtr")
                            nc.tensor.transpose(
                                pt[:], kvasm[:, 2 + h, st * P:(st + 1) * P],
                                ident_r[:])
                            nc.vector.tensor_copy(v_tok[:, h, st], pt[:])
                # q blocks (after AG1b), rope applied inline
                for blk in range(8):
                    sl = slice(blk * SS, (blk + 1) * SS)
                    rhs_u = sbr.tile([P, 12, SS], BF16, tag="rhs1")
                    nc.sync.dma_start(
                        rhs_u[:],
                        ag1b_out[blk * P:(blk + 1) * P, :].rearrange(
                            "p (kt s) -> p kt s", s=SS))
                    qpe_b = None
                    for mc in range(3):
                        ps = pbs.tile([P, SS], F32, tag="qb")
                        for kt in range(12):
                            nc.tensor.matmul(ps[:], wqb[:, kt, mc], rhs_u[:, kt],
                                             start=(kt == 0), stop=(kt == 11))
                        if mc < 2:
                            nc.vector.tensor_copy(qasm[:, mc, sl], ps[:])
                        else:
                            qpe_b = sbt.tile([P, SS], F32, tag="qpe")
                            nc.vector.tensor_copy(qpe_b[:], ps[:])
                    qsw = sbt.tile([P, SS], F32, tag="qsw")
                    for bb, sb in [(0, 1), (1, 0), (2, 3), (3, 2)]:
                        nc.sync.dma_start(qsw[32 * bb:32 * bb + 32, :],
                                          qpe_b[32 * sb:32 * sb + 32, :])
                    t1 = sbt.tile([P, SS], F32, tag="t1")
                    nc.vector.tensor_mul(t1[:], qpe_b[:], cos_t[:, sl])
                    t2 = sbt.tile([P, SS], F32, tag="t2")
                    nc.vector.tensor_mul(t2[:], qsw[:], sin_t[:, sl])
                    qrot = sbt.tile([P, SS], BF16, tag="qrot")
                    nc.vector.tensor_add(qrot[:], t1[:], t2[:])
                    nc.scalar.dma_start(qpe2[:, 0, sl], qrot[0:64, :])
                    nc.scalar.dma_start(qpe2[:, 1, sl], qrot[64:128, :])

                if DEBUG:
                    nc.sync.dma_start(dbg["dbg_kT"][:, :, :], kvasm[:, 0:2])
                    nc.sync.dma_start(dbg["dbg_qT"][:, :, :], qasm[:, :])

        # ================= attention + pipelined o_proj/post-LN =================
        with tc.tile_pool(name="sbe", bufs=6) as sbe, \
             tc.tile_pool(name="sbm", bufs=1) as sbm, \
             tc.tile_pool(name="scs", bufs=1) as scs, \
             tc.tile_pool(name="scr", bufs=1) as scr, \
             tc.tile_pool(name="pat", bufs=1, space="PSUM") as pat:
            wos = sbm.tile([P, 16, 2, P], BF16)
            nc.sync.dma_start(wos[:], wo_t.rearrange("a b p m -> p a b m"))

            def cproj_a(j):
                """o_proj chunk j + residual + sq-stats + AR4_j."""
                nsl = slice(j * 512, (j + 1) * 512)
                rhs = scr.tile([P, 16, 512], BF16, tag="rhs2", name="rhs2")
                nc.sync.dma_start(
                    rhs[:], ag2_out[j].rearrange("(kt p) s -> p kt s", p=P))
                resid = scs.tile([P, 2, 512], F32, tag="resid", name="resid")
                nc.sync.dma_start(
                    resid[:],
                    hT_r.rearrange("(mc p) s -> p mc s", p=P)[:, :, nsl])
                sqh = scs.tile([P, 2, 512], BF16, tag="sqh", name="sqh")
                for mc in range(2):
                    ps = pat.tile([P, 512], F32, tag="rb", bufs=1, name="omm")
                    for kt in range(16):
                        nc.tensor.matmul(ps[:], wos[:, kt, mc], rhs[:, kt],
                                         start=(kt == 0), stop=(kt == 15))
                    nc.vector.tensor_add(h2[:, mc, nsl], ps[:], resid[:, mc])
                    nc.vector.tensor_mul(sqh[:, mc], h2[:, mc, nsl],
                                         h2[:, mc, nsl])
                ps4 = pat.tile([1, 512], F32, tag="m4", bufs=1, name="m4")
                for mc in range(2):
                    nc.tensor.matmul(ps4[:], ones_r[:], sqh[:, mc],
                                     start=(mc == 0), stop=(mc == 1))
                msq4 = scs.tile([1, 512], F32, tag="msq4", name="msq4")
                nc.vector.tensor_copy(msq4[:], ps4[:])
                nc.sync.dma_start(ar4_in[j][:, :], msq4[:])
                nc.gpsimd.collective_compute(
                    "AllReduce", mybir.AluOpType.add, replica_groups=RG,
                    ins=[ar4_in[j]], outs=[ar4_out[j]])

            def cproj_b(j):
                """r4_j + yT_j + AG3_j."""
                nsl = slice(j * 512, (j + 1) * 512)
                msq4g = scs.tile([1, 512], F32, tag="m4g", name="msq4g")
                nc.sync.dma_start(msq4g[:], ar4_out[j][:, :])
                r4s = scs.tile([1, 512], F32, tag="r4s", name="r4s")
                nc.scalar.activation(r4s[:], msq4g[:], AF.Sqrt,
                                     scale=1.0 / H, bias=eps_t[:1])
                r4sr = scs.tile([1, 512], F32R, tag="r4sr", name="r4sr")
                nc.vector.tensor_copy(r4sr[:], r4s[:])
                r4bp = pat.tile([P, 512], F32, tag="rb", bufs=1, name="rb")
                nc.tensor.matmul(r4bp[:], ones_k1r[:], r4sr[:],
                                 start=True, stop=True)
                r4b = scs.tile([P, 512], F32, tag="r4b", name="r4b")
                nc.vector.reciprocal_approx_fast(r4b[:], r4bp[:])
                yT = scs.tile([P, 2, 512], BF16, tag="yT", name="yT")
                nc.vector.tensor_mul(
                    yT[:], h2[:, :, nsl],
                    r4b[:, None, :].to_broadcast([P, 2, 512]))
                nc.sync.dma_start(
                    ag3_in[j].rearrange("(mc p) s -> p mc s", p=P), yT[:])
                nc.gpsimd.collective_compute(
                    "AllGather", mybir.AluOpType.bypass, replica_groups=RG,
                    ins=[ag3_in[j]], outs=[ag3_out[j]])

            with nc.named_scope("stageB_attn"):
                mask_t = sbm.tile([P, 4, 512], BF16)
                nc.sync.dma_start(mask_t[:], dmask[:, :, :])
                for qc in range(4):
                    qsl = slice(qc * 512, (qc + 1) * 512)
                    nkt = 4 * qc + 4
                    o_ps = [pat.tile([P, 512], F32, tag="o", bufs=2,
                                     name=f"o{h}") for h in range(2)]
                    d_ps = [pat.tile([1, 512], F32, tag="d", bufs=2,
                                     name=f"d{h}") for h in range(2)]
                    for kt in range(nkt):
                        ksl = slice(kt * P, (kt + 1) * P)
                        j = kt - 4 * qc
                        for h in range(2):
                            sc_ps = pat.tile([P, 512], F32, tag="sc", bufs=2,
                                             name="scp")
                            nc.tensor.matmul(sc_ps[:], kvasm[:, h, ksl],
                                             qasm[:, h, qsl], start=True,
                                             stop=False)
                            nc.tensor.matmul(sc_ps[:], kpeT[:, ksl],
                                             qpe2[:, h, qsl], start=False,
                                             stop=True)
                            if j >= 0:
                                nc.vector.tensor_add(sc_ps[:], sc_ps[:],
                                                     mask_t[:, j])
                            es = sbe.tile([P, 512], BF16, tag="es", bufs=4)
                            nc.scalar.activation(es[:], sc_ps[:], AF.Exp)
                            nc.tensor.matmul(o_ps[h][:], v_tok[:, h, kt], es[:],
                                             start=(kt == 0), stop=(kt == nkt - 1))
                            nc.tensor.matmul(d_ps[h][:], ones_r[:], es[:],
                                             start=(kt == 0), stop=(kt == nkt - 1))
                    for h in range(2):
                        ds = sbe.tile([1, 512], F32R, tag="ds", bufs=2)
                        nc.vector.tensor_copy(ds[:], d_ps[h][:])
                        rb_ps = pat.tile([P, 512], F32, tag="rb", bufs=1,
                                         name="rbo")
                        nc.tensor.matmul(rb_ps[:], ones_k1r[:], ds[:],
                                         start=True, stop=True)
                        recb = sbe.tile([P, 512], F32, tag="recb", bufs=2)
                        nc.vector.reciprocal_approx_fast(recb[:], rb_ps[:])
                        nc.vector.tensor_mul(oT[:, h, qsl], o_ps[h][:], recb[:])
                    nc.sync.dma_start(
                        ag2_in[qc].rearrange("(mc p) s -> p mc s", p=P),
                        oT[:, :, qsl])
                    nc.gpsimd.collective_compute(
                        "AllGather", mybir.AluOpType.bypass, replica_groups=RG,
                        ins=[ag2_in[qc]], outs=[ag2_out[qc]])
                    if qc >= 1:
                        cproj_a(qc - 1)
                    if qc >= 2:
                        cproj_b(qc - 2)
                cproj_a(3)
                cproj_b(2)
                cproj_b(3)
                if DEBUG:
                    nc.sync.dma_start(dbg["dbg_oT"][:, :, :], oT[:])
                    nc.sync.dma_start(dbg["dbg_h2"][:, :, :], h2[:])

        bpool_cm.__exit__(None, None, None)

        # ================= Stage D: merged MLP pipeline =================
        with tc.tile_pool(name="sd", bufs=1) as sd, \
             tc.tile_pool(name="sdr", bufs=2) as sdr, \
             tc.tile_pool(name="sde", bufs=4) as sde, \
             tc.tile_pool(name="sdd", bufs=6) as sdd, \
             tc.tile_pool(name="pdg", bufs=2, space="PSUM") as pdg:
            with nc.named_scope("stageD"):
                wds = sd.tile([P, 8, 16, P], BF16)
                for mc in range(16):
                    nc.gpsimd.dma_start(wds[:, :, mc, :],
                                        wd_t[:, mc].rearrange("a p m -> p a m"))
                act = sd.tile([P, 8, 512], BF16, name="act")  # per-ncol activations

                # rs chunk layout: (ncol, col offset within ncol, width, rs idx)
                CH = {0: [(0, 512, 0)], 1: [(0, 512, 1)], 2: [(0, 512, 2)],
                      3: [(0, 256, 3), (256, 256, 4)]}
                for ncol in range(4):
                    nsl = slice(ncol * 512, (ncol + 1) * 512)
                    rhs = sdr.tile([P, 16, 512], BF16, tag="rhs3")
                    nc.sync.dma_start(
                        rhs[:], ag3_out[ncol].rearrange("(kt p) s -> p kt s", p=P))
                    for m in range(8):
                        gp = pdg.tile([P, 512], F32, tag="g", name="gps")
                        up = pdg.tile([P, 512], F32, tag="u", name="ups")
                        for kt in range(16):
                            nc.tensor.matmul(gp[:], wgs[:, kt, m], rhs[:, kt],
                                             start=(kt == 0), stop=(kt == 15))
                            nc.tensor.matmul(up[:], wus[:, kt, m], rhs[:, kt],
                                             start=(kt == 0), stop=(kt == 15))
                        gsil = sde.tile([P, 512], BF16, tag="gsil")
                        nc.scalar.activation(gsil[:], gp[:], AF.Silu)
                        nc.vector.tensor_mul(act[:, m], gsil[:], up[:])
                    # down projection for this token-column block
                    for (c0, cw, jr) in CH[ncol]:
                        csl = slice(c0, c0 + cw)
                        gsl = slice(ncol * 512 + c0, ncol * 512 + c0 + cw)
                        for mc in range(16):
                            ps = pdg.tile([P, 512], F32, tag="dmm",
                                          name="dmmps")[:, :cw]
                            for kt in range(8):
                                nc.tensor.matmul(ps[:], wds[:, kt, mc],
                                                 act[:, kt, csl],
                                                 start=(kt == 0), stop=(kt == 7))
                            dn = sdd.tile([P, 512], BF16, tag="dn",
                                          name="dntile")[:, :cw]
                            if mc % 2 == 0:
                                nc.vector.tensor_copy(dn[:], ps[:])
                            else:
                                nc.scalar.activation(dn[:], ps[:], AF.Copy)
                            nc.sync.dma_start(rs_in[jr][mc * P:(mc + 1) * P, :],
                                              dn[:])
                        nc.gpsimd.collective_compute(
                            "ReduceScatter", mybir.AluOpType.add,
                            replica_groups=RG,
                            ins=[rs_in[jr]], outs=[rs_out[jr]])
                        fin = sdd.tile([P, 2, 512], BF16, tag="fin", bufs=2,
                                       name="fintile")[:, :, :cw]
                        nc.sync.dma_start(
                            fin[:],
                            rs_out[jr].rearrange("(mc p) s -> p mc s", p=P))
                        fino = sdd.tile([P, 2, 512], F32, tag="fino", bufs=2,
                                        name="finotile")[:, :, :cw]
                        nc.vector.tensor_add(fino[:], fin[:], h2[:, :, gsl])
                        nc.sync.dma_start(
                            outT.rearrange("(mc p) s -> p mc s", p=P)[:, :, gsl],
                            fino[:])

    nc.compile()
    _CACHE["nc"] = nc
    return nc


def _host_prep(inputs):
    import ml_dtypes
    bf16 = ml_dtypes.bfloat16
    inp = {k: np.asarray(v) for k, v in inputs.items()}
    hidden = inp["hidden_states"].reshape(S, H).astype(np.float32)
    pos = inp["position_ids"].reshape(S).astype(np.int64)
    cosT = inp["cos"][pos].T.astype(np.float32)
    sinT = inp["sin"][pos].T.astype(np.float32)
    wq_a = (inp["wq_a"] * inp["in_ln"][:, None]).astype(np.float32)
    wkv_a = (inp["wkv_a"] * inp["in_ln"][:, None]).astype(np.float32)
    wq_b = (inp["wq_b"] * inp["q_a_ln"][:, None]).astype(np.float32)
    wkv_b = (inp["wkv_b"] * inp["kv_a_ln"][:, None]).astype(np.float32)
    wg = (inp["w_gate"] * inp["post_ln"][:, None]).astype(np.float32)
    wu = (inp["w_up"] * inp["post_ln"][:, None]).astype(np.float32)
    wd = inp["w_down"].astype(np.float32)
    wo = inp["wo"].astype(np.float32)

    de = np.empty(ROPE, np.int64)
    de[:32] = np.arange(32) * 2
    de[32:] = np.arange(32) * 2 + 1
    wkv_a = np.concatenate([wkv_a[:, :KVLR], wkv_a[:, KVLR:][:, de]], axis=1)
    wq_b = wq_b.reshape(QLR, NH, QHD)
    wkv_b = wkv_b.reshape(KVLR, NH, NOPE + VHD)

    hT = hidden.T.copy()
    sin_sg = np.concatenate([-sinT[:32], sinT[32:]], axis=0)    # signed for swap trick
    cossin = np.concatenate([cosT, cosT, sin_sg, sin_sg], axis=0)  # (256, S)
    ki = np.arange(P)[:, None]
    qi = np.arange(512)[None, :]
    dmask = np.stack([np.where(qi >= j * P + ki, 0.0, -1e30).astype(np.float32)
                      for j in range(4)], axis=1).astype(bf16)  # (128, 4, 512)

    wq_a_t = _tile_w(wq_a).astype(bf16)
    wkv_a_t = _tile_w(wkv_a).astype(bf16)

    in_maps = []
    for c in range(NC):
        h0, h1 = 2 * c, 2 * c + 1
        qb = np.concatenate([
            wq_b[:, h0, :NOPE], wq_b[:, h1, :NOPE],
            wq_b[:, h0, NOPE:][:, de], wq_b[:, h1, NOPE:][:, de]], axis=1) * SCALE
        kb = np.concatenate([
            wkv_b[:, h0, :NOPE], wkv_b[:, h1, :NOPE],
            wkv_b[:, h0, NOPE:], wkv_b[:, h1, NOPE:]], axis=1)
        ssl = slice(c * SS, (c + 1) * SS)
        cs_sh = np.concatenate([cosT[:, ssl], sin_sg[:, ssl]], axis=0)
        in_maps.append({
            "hT_s": np.ascontiguousarray(hT[:, ssl]),
            "hT_r": np.ascontiguousarray(hT[ssl, :]),
            "wq_a_t": wq_a_t,
            "wkv_a_t": wkv_a_t,
            "wq_b_t": _tile_w(qb.astype(np.float32)).astype(bf16),
            "wkv_b_t": _tile_w(kb.astype(np.float32)).astype(bf16),
            "wo_t": _tile_w(np.ascontiguousarray(wo[:, ssl])).astype(bf16),
            "wg_t": _tile_w(wg[:, c * FFS:(c + 1) * FFS]).astype(bf16),
            "wu_t": _tile_w(wu[:, c * FFS:(c + 1) * FFS]).astype(bf16),
            "wd_t": _tile_w(wd[c * FFS:(c + 1) * FFS, :]).astype(bf16),
            "cossin": cossin.astype(bf16),
            "cs_sh": np.ascontiguousarray(cs_sh),
            "dmask": dmask,
        })
    return in_maps


_LAST_RESULT = {}


def kernel(**inputs) -> np.ndarray:
    from concourse.bass_utils import run_bass_kernel_spmd
    nc = _build()
    in_maps = _host_prep(inputs)
    kwargs = {}
    if TRACE:
        import sys, types
        if "antenv.axon_hooks" not in sys.modules:
            try:
                from trn_agent_boot.trn_boot import _ntff_profile_via_ctypes
                mod = types.ModuleType("antenv.axon_hooks")
                _hook = _ntff_profile_via_ctypes('/opt/axon/libaxon_pjrt.so')
                mod.get_axon_ntff_profile_hook = lambda: _hook
                mod.set_axon_ntff_profile_hook = lambda h: None
                sys.modules["antenv.axon_hooks"] = mod
                import antenv
                antenv.axon_hooks = mod
            except Exception:
                pass
        kwargs["trace"] = True
    res = run_bass_kernel_spmd(nc, in_maps, list(range(NC)), **kwargs)
    _LAST_RESULT["res"] = res
    outT = np.concatenate([res.results[c]["outT"] for c in range(NC)], axis=0)
    return np.ascontiguousarray(outT.T)[None].astype(np.float32)


# revision 25
# speedup vs baseline: 1.0523x; 1.0378x over previous
"""DeepseekV3 decoder layer on 8 Trainium2 NeuronCores (Bass/Tile), v2.

Sharding: token-sharded low-rank path with LOCAL latents (no latent
AllGather): each core computes q/k/v for ALL 16 heads on its own 256
tokens, then small AllToAlls redistribute tokens->heads for attention
(2 heads/core over all tokens).  k_pe (MQA, shared) uses a tiny
AllGather.  Attention uses transposed-score layout, head-interleaved
kt loop, exp-sum accumulated on the Vector engine (one f32r matmul per
(qc,head) for the denominator), and fast approximate reciprocals.
o_proj stays output-feature-sharded fed by a per-q-chunk AllGather,
with post-LN stats via tiny AllReduce, AllGather of the normed MLP
input, and a merged FF-sharded MLP pipeline (gate/up/down + chunked
ReduceScatter per token-column block) with small tail chunks.

RMS/ln weights and the rope de-interleave are folded into the weight
matrices host-side.  RMS normalization scales are folded into the
projection OUTPUTS (projections run on raw x), removing the serial
stats->matmul dependency at stage starts.
"""

import numpy as np

B, S, H = 1, 2048, 2048
NH, NOPE, ROPE, VHD = 16, 128, 64, 128
QHD = NOPE + ROPE
QLR, KVLR, FF = 1536, 512, 8192
SCALE = QHD ** -0.5
EPS = 1e-6
NC = 8
SS = S // NC            # 256: token shard
FFS = FF // NC          # 1024: FF shard
P = 128

TRACE = False           # test.py sets kernel.TRACE = True for profiling
DEBUG = False

_CACHE = {}


def _tile_w(w):
    """[K, M] -> [K/128, ceil(M/128), 128, 128] contiguous blocks (zero-pad M)."""
    K, M = w.shape
    mc = -(-M // P)
    out = np.zeros((K // P, mc, P, P), np.float32)
    wp = np.zeros((K, mc * P), np.float32)
    wp[:, :M] = w
    for kt in range(K // P):
        for m in range(mc):
            out[kt, m] = wp[kt * P:(kt + 1) * P, m * P:(m + 1) * P]
    return out


def _build():
    if "nc" in _CACHE:
        return _CACHE["nc"]
    import concourse.mybir as mybir
    import concourse.tile as tile
    from concourse import bacc
    from concourse.masks import make_identity

    F32 = mybir.dt.float32
    F32R = mybir.dt.float32r
    BF16 = mybir.dt.bfloat16
    AF = mybir.ActivationFunctionType

    nc = bacc.Bacc("TRN2", target_bir_lowering=False, debug=False, num_devices=NC)

    def inp(name, shape, dt=F32):
        return nc.dram_tensor(name, list(shape), dt, kind="ExternalInput").ap()

    hT_s = inp("hT_s", [H, SS])               # own tokens, all features
    hT_r = inp("hT_r", [SS, S])               # own features, all tokens (resid)
    wq_a_t = inp("wq_a_t", [16, 12, P, P], BF16)
    wkv_a_t = inp("wkv_a_t", [16, 5, P, P], BF16)
    wq_b_t = inp("wq_b_t", [12, 3, P, P], BF16)    # own 2 heads
    wkv_b_t = inp("wkv_b_t", [4, 4, P, P], BF16)   # own 2 heads
    wo_t = inp("wo_t", [16, 2, P, P], BF16)
    wg_t = inp("wg_t", [16, 8, P, P], BF16)
    wu_t = inp("wu_t", [16, 8, P, P], BF16)
    wd_t = inp("wd_t", [8, 16, P, P], BF16)
    cossin = inp("cossin", [2 * P, S], BF16)        # rows 0:128 [cosT;cosT], 128:256 [sinT;sinT]
    cs_sh = inp("cs_sh", [P, SS])             # rows 0:64 cosT, 64:128 signed sinT (own shard)
    dmask = inp("dmask", [P, 4, 512], BF16)
    outT = nc.dram_tensor("outT", [SS, S], F32, kind="ExternalOutput").ap()

    RG = [list(range(NC))]
    dbg = {}
    if DEBUG:
        for nm, shp, dt in [("dbg_un", [P, 12, SS], BF16),
                            ("dbg_ckn", [P, 4, SS], BF16),
                            ("dbg_kT", [P, 2, S], BF16),
                            ("dbg_qT", [P, 2, S], BF16),
                            ("dbg_oT", [P, 2, S], BF16),
                            ("dbg_h2", [P, 2, S], F32)]:
            dbg[nm] = nc.dram_tensor(nm, shp, dt, kind="ExternalOutput").ap()

    from contextlib import ExitStack
    with tile.TileContext(nc) as tc, ExitStack() as _stack:
        cpool = _stack.enter_context(tc.tile_pool(name="const", bufs=1))
        dpool = _stack.enter_context(tc.tile_pool(name="dram", bufs=1, space="DRAM"))
        ppool = _stack.enter_context(tc.tile_pool(name="persist", bufs=1))

        # ---- DRAM collective buffers ----
        ag1a_in = dpool.tile([P, 5 * SS], BF16)
        ag1a_out = dpool.tile([NC * P, 5 * SS], BF16, addr_space="Shared")
        ag1b_in = dpool.tile([P, 12 * SS], BF16)
        ag1b_out = dpool.tile([NC * P, 12 * SS], BF16, addr_space="Shared")
        ag2_in = [dpool.tile([2 * VHD, 512], BF16, name=f"ag2_in{j}")
                  for j in range(4)]
        ag2_out = [dpool.tile([NH * VHD, 512], BF16, addr_space="Shared",
                              name=f"ag2_out{j}") for j in range(4)]
        ar4_in = [dpool.tile([1, 512], F32, name=f"ar4_in{j}") for j in range(4)]
        ar4_out = [dpool.tile([1, 512], F32, addr_space="Shared",
                              name=f"ar4_out{j}") for j in range(4)]
        ag3_in = [dpool.tile([SS, 512], BF16, name=f"ag3_in{j}") for j in range(4)]
        ag3_out = [dpool.tile([H, 512], BF16, addr_space="Shared",
                              name=f"ag3_out{j}") for j in range(4)]
        _rs_w = [512, 512, 512, 256, 256]
        rs_in = [dpool.tile([H, _rs_w[j]], BF16, name=f"rs_in{j}") for j in range(5)]
        rs_out = [dpool.tile([SS, _rs_w[j]], BF16, name=f"rs_out{j}")
                  for j in range(5)]

        # ---- constants ----
        ones_f = cpool.tile([P, 1], F32)
        nc.vector.memset(ones_f[:], 1.0)
        ones_r = cpool.tile([P, 1], BF16)
        nc.vector.tensor_copy(ones_r[:], ones_f[:])
        ident_f = cpool.tile([P, P], F32)
        make_identity(nc, ident_f)
        ident_r = cpool.tile([P, P], BF16)
        nc.vector.tensor_copy(ident_r[:], ident_f[:])
        eps_t = cpool.tile([P, 1], F32)
        nc.vector.memset(eps_t[:], EPS)
        ones_k1f = cpool.tile([1, P], F32)
        nc.vector.memset(ones_k1f[:], 1.0)
        ones_k1r = cpool.tile([1, P], F32R)
        nc.vector.tensor_copy(ones_k1r[:], ones_k1f[:])
        ones_pr = cpool.tile([P, 1], F32R)
        nc.vector.tensor_copy(ones_pr[:], ones_f[:])

        def r32(ap):
            return ap.bitcast(F32R)

        h2 = ppool.tile([P, 2, S], F32)
        # carried from stage A into qkv
        un = ppool.tile([P, 12, SS], BF16)
        ckn = ppool.tile([P, 4, SS], BF16)

        # gate/up weights: loaded during qkv/attention (gpsimd-queue DMAs)
        wpool = _stack.enter_context(tc.tile_pool(name="mlpw", bufs=1))
        wgs = wpool.tile([P, 16, 8, P], BF16)
        wus = wpool.tile([P, 16, 8, P], BF16)

        # ================= Stage A: local low-rank path =================
        with tc.tile_pool(name="sa", bufs=1) as sa, \
             tc.tile_pool(name="saw", bufs=4) as saw, \
             tc.tile_pool(name="sas", bufs=3) as sas, \
             tc.tile_pool(name="pa", bufs=2, space="PSUM") as pa:
            with nc.named_scope("stageA"):
                xs = sa.tile([P, 16, SS], F32)
                nc.sync.dma_start(xs[:], hT_s.rearrange("(kt p) s -> p kt s", p=P))
                xb = sa.tile([P, 16, SS], BF16)
                for c in range(4):
                    nc.vector.tensor_copy(xb[:, 4 * c:4 * c + 4],
                                          xs[:, 4 * c:4 * c + 4])

                # all stage-A weight DMAs first (scalar queue, ahead of ACT ops)
                kvw = []
                qw = []
                for mc in range(5):
                    wt = saw.tile([P, 16, P], BF16, tag="kva", bufs=5)
                    nc.scalar.dma_start(wt[:], wkv_a_t[:, mc].rearrange("a p m -> p a m"))
                    kvw.append(wt)
                for mc in range(12):
                    wt = saw.tile([P, 16, P], BF16, tag="aw", bufs=6)
                    nc.scalar.dma_start(wt[:], wq_a_t[:, mc].rearrange("a p m -> p a m"))
                    qw.append(wt)

                # x stats (chunked squares feed accumulating ones-matmuls)
                msq1 = pa.tile([1, SS], F32, tag="msq", name="msq1")
                for c in range(4):
                    sqc = sas.tile([P, 4, SS], BF16, tag="sq")
                    nc.vector.tensor_mul(sqc[:], xs[:, 4 * c:4 * c + 4],
                                         xs[:, 4 * c:4 * c + 4])
                    for k in range(4):
                        nc.tensor.matmul(msq1[:], ones_r[:], sqc[:, k],
                                         start=(c == 0 and k == 0),
                                         stop=(c == 3 and k == 3))

                # raw kv projections first (short path -> AG1a early)
                cvs = sa.tile([P, 5, SS], F32)
                for mc in range(5):
                    ps = pa.tile([P, SS], F32, tag="amm")
                    for kt in range(16):
                        nc.tensor.matmul(ps[:], kvw[mc][:, kt], xb[:, kt],
                                         start=(kt == 0), stop=(kt == 15))
                    nc.vector.tensor_copy(cvs[:, mc], ps[:])

                r1s = sa.tile([1, SS], F32)
                nc.scalar.activation(r1s[:], msq1[:], AF.Sqrt, scale=1.0 / H,
                                     bias=eps_t[:1])
                r1sr = sa.tile([1, SS], F32R)
                nc.vector.tensor_copy(r1sr[:], r1s[:])
                r1bp = pa.tile([P, SS], F32, tag="rb", name="r1bp")
                nc.tensor.matmul(r1bp[:], ones_k1r[:], r1sr[:],
                                 start=True, stop=True)
                r1b = sa.tile([P, SS], F32)
                nc.vector.reciprocal_approx_fast(r1b[:], r1bp[:])
                r1sq = sa.tile([P, SS], F32)
                nc.vector.tensor_mul(r1sq[:], r1b[:], r1b[:])

                # kv latent stats on raw cvs; fold r1 into the scale
                sq3 = sas.tile([P, 4, SS], BF16, tag="sq")
                nc.vector.tensor_mul(sq3[:], cvs[:, :4], cvs[:, :4])
                msq3 = pa.tile([1, SS], F32, tag="msq", name="msq3")
                for k in range(4):
                    nc.tensor.matmul(msq3[:], ones_r[:], sq3[:, k],
                                     start=(k == 0), stop=(k == 3))
                msq3s = sa.tile([1, SS], F32R)
                nc.vector.tensor_copy(msq3s[:], msq3[:])
                m3bp = pa.tile([P, SS], F32, tag="rb", name="m3bp")
                nc.tensor.matmul(m3bp[:], ones_k1r[:], msq3s[:],
                                 start=True, stop=True)
                m3s = sa.tile([P, SS], F32)
                nc.vector.tensor_mul(m3s[:], m3bp[:], r1sq[:])
                r3s = sa.tile([P, SS], F32)
                nc.scalar.activation(r3s[:], m3s[:], AF.Sqrt, scale=1.0 / KVLR,
                                     bias=eps_t[:])
                r3b = sa.tile([P, SS], F32)
                nc.vector.reciprocal_approx_fast(r3b[:], r3s[:])
                s3 = sa.tile([P, SS], F32)
                nc.vector.tensor_mul(s3[:], r1b[:], r3b[:])
                nc.vector.tensor_mul(ckn[:], cvs[:, :4],
                                     s3[:, None, :].to_broadcast([P, 4, SS]))

                # k_pe rope on raw cvs[:, 4], scaled by r1
                cos_sh = sa.tile([64, SS], F32)
                nc.sync.dma_start(cos_sh[:], cs_sh[0:64, :])
                sin_sh = sa.tile([64, SS], F32)
                nc.sync.dma_start(sin_sh[:], cs_sh[64:128, :])
                ksw = sa.tile([64, SS], F32)
                nc.sync.dma_start(ksw[0:32, :], cvs[32:64, 4])
                nc.sync.dma_start(ksw[32:64, :], cvs[0:32, 4])
                kp1 = sa.tile([64, SS], F32)
                nc.vector.tensor_mul(kp1[:], cvs[:64, 4], cos_sh[:])
                kp2 = sa.tile([64, SS], F32)
                nc.vector.tensor_mul(kp2[:], ksw[:], sin_sh[:])
                nc.vector.tensor_add(kp1[:], kp1[:], kp2[:])
                kpe_n = sa.tile([64, SS], BF16)
                nc.vector.tensor_mul(kpe_n[:], kp1[:], r1b[:64, :])
                nc.sync.dma_start(
                    ag1a_in[:, 0:4 * SS].rearrange("p (kt s) -> p kt s", s=SS),
                    ckn[:])
                nc.sync.dma_start(ag1a_in[:64, 4 * SS:5 * SS], kpe_n[:])
                nc.gpsimd.collective_compute(
                    "AllGather", mybir.AluOpType.bypass, replica_groups=RG,
                    ins=[ag1a_in], outs=[ag1a_out])

                # raw q projections
                us = sa.tile([P, 12, SS], F32)
                msq2 = pa.tile([1, SS], F32, tag="msq", name="msq2")
                for mc in range(12):
                    ps = pa.tile([P, SS], F32, tag="amm")
                    for kt in range(16):
                        nc.tensor.matmul(ps[:], qw[mc][:, kt], xb[:, kt],
                                         start=(kt == 0), stop=(kt == 15))
                    nc.vector.tensor_copy(us[:, mc], ps[:])
                    sq2 = sas.tile([P, SS], BF16, tag="sq2")
                    nc.vector.tensor_mul(sq2[:], us[:, mc], us[:, mc])
                    nc.tensor.matmul(msq2[:], ones_r[:], sq2[:],
                                     start=(mc == 0), stop=(mc == 11))
                msq2s = sa.tile([1, SS], F32R)
                nc.vector.tensor_copy(msq2s[:], msq2[:])
                m2bp = pa.tile([P, SS], F32, tag="rb", name="m2bp")
                nc.tensor.matmul(m2bp[:], ones_k1r[:], msq2s[:],
                                 start=True, stop=True)
                m2s = sa.tile([P, SS], F32)
                nc.vector.tensor_mul(m2s[:], m2bp[:], r1sq[:])
                r2s = sa.tile([P, SS], F32)
                nc.scalar.activation(r2s[:], m2s[:], AF.Sqrt, scale=1.0 / QLR,
                                     bias=eps_t[:])
                r2b = sa.tile([P, SS], F32)
                nc.vector.reciprocal_approx_fast(r2b[:], r2s[:])
                s2 = sa.tile([P, SS], F32)
                nc.vector.tensor_mul(s2[:], r1b[:], r2b[:])
                nc.vector.tensor_mul(un[:], us[:],
                                     s2[:, None, :].to_broadcast([P, 12, SS]))
                nc.sync.dma_start(
                    ag1b_in.rearrange("p (kt s) -> p kt s", s=SS), un[:])
                nc.gpsimd.collective_compute(
                    "AllGather", mybir.AluOpType.bypass, replica_groups=RG,
                    ins=[ag1b_in], outs=[ag1b_out])
                for m in range(8):
                    nc.vector.tensor_copy(wgs[0:1, 0:1, m, 0:1],
                                          ckn[0:1, 0:1, 0:1])
                    nc.vector.tensor_copy(wus[0:1, 0:1, m, 0:1],
                                          ckn[0:1, 0:1, 0:1])
                    nc.gpsimd.dma_start(wgs[:, :, m, :],
                                        wg_t[:, m].rearrange("a p m -> p a m"))
                    nc.gpsimd.dma_start(wus[:, :, m, :],
                                        wu_t[:, m].rearrange("a p m -> p a m"))
                if DEBUG:
                    nc.sync.dma_start(dbg["dbg_un"][:, :, :], un[:])
                    nc.sync.dma_start(dbg["dbg_ckn"][:, :, :], ckn[:])

        # attention-phase tiles (freed before stage D)
        bpool_cm = tc.tile_pool(name="battn", bufs=1)
        bpool = bpool_cm.__enter__()
        kvasm = bpool.tile([P, 4, S], BF16)    # [kT_h0 | kT_h1 | v_h0 | v_h1]
        v_tok = bpool.tile([P, 2, 16, P], BF16)
        qasm = bpool.tile([P, 2, S], BF16)     # [qT_h0 | qT_h1]
        qpe2 = bpool.tile([64, 2, S], BF16)
        kpeT = bpool.tile([64, S], BF16)
        oT = bpool.tile([P, 2, S], BF16)

        # ============ Stage Bq: per-2-head q/k/v from gathered latents ============
        with tc.tile_pool(name="sbw", bufs=1) as sbw, \
             tc.tile_pool(name="sbr", bufs=3) as sbr, \
             tc.tile_pool(name="sbt", bufs=2) as sbt, \
             tc.tile_pool(name="pbs", bufs=2, space="PSUM") as pbs:
            with nc.named_scope("stageB_qkv"):
                cos_t = sbw.tile([P, S], BF16)
                nc.sync.dma_start(cos_t[:], cossin[0:P, :])
                sin_t = sbw.tile([P, S], BF16)
                nc.sync.dma_start(sin_t[:], cossin[P:2 * P, :])
                wqb = sbw.tile([P, 12, 3, P], BF16)
                nc.scalar.dma_start(wqb[:], wq_b_t.rearrange("a b p m -> p a b m"))
                wkb = sbw.tile([P, 4, 4, P], BF16)
                nc.scalar.dma_start(wkb[:], wkv_b_t.rearrange("a b p m -> p a b m"))

                # kv blocks first (AG1a lands early) + V transposes
                for blk in range(8):
                    sl = slice(blk * SS, (blk + 1) * SS)
                    rhs_c = sbr.tile([P, 4, SS], BF16, tag="rhs1c")
                    nc.sync.dma_start(
                        rhs_c[:],
                        ag1a_out[blk * P:(blk + 1) * P, 0:4 * SS].rearrange(
                            "p (kt s) -> p kt s", s=SS))
                    for mc in range(4):
                        ps = pbs.tile([P, SS], F32, tag="qb")
                        for kt in range(4):
                            nc.tensor.matmul(ps[:], wkb[:, kt, mc], rhs_c[:, kt],
                                             start=(kt == 0), stop=(kt == 3))
                        nc.vector.tensor_copy(kvasm[:, mc, sl], ps[:])
                    nc.sync.dma_start(
                        kpeT[:, sl],
                        ag1a_out[blk * P:blk * P + 64, 4 * SS:5 * SS])
                    for h in range(2):
                        for st in (2 * blk, 2 * blk + 1):
                            pt = pbs.tile([P, P], BF16, tag="vtr")
                            nc.tensor.transpose(
                                pt[:], kvasm[:, 2 + h, st * P:(st + 1) * P],
                                ident_r[:])
                            nc.vector.tensor_copy(v_tok[:, h, st], pt[:])
                # q blocks (after AG1b), rope applied inline
                for blk in range(8):
                    sl = slice(blk * SS, (blk + 1) * SS)
                    rhs_u = sbr.tile([P, 12, SS], BF16, tag="rhs1")
                    nc.sync.dma_start(
                        rhs_u[:],
                        ag1b_out[blk * P:(blk + 1) * P, :].rearrange(
                            "p (kt s) -> p kt s", s=SS))
                    qpe_b = None
                    for mc in range(3):
                        ps = pbs.tile([P, SS], F32, tag="qb")
                        for kt in range(12):
                            nc.tensor.matmul(ps[:], wqb[:, kt, mc], rhs_u[:, kt],
                                             start=(kt == 0), stop=(kt == 11))
                        if mc < 2:
                            nc.vector.tensor_copy(qasm[:, mc, sl], ps[:])
                        else:
                            qpe_b = sbt.tile([P, SS], F32, tag="qpe")
                            nc.vector.tensor_copy(qpe_b[:], ps[:])
                    qsw = sbt.tile([P, SS], F32, tag="qsw")
                    for bb, sb in [(0, 1), (1, 0), (2, 3), (3, 2)]:
                        nc.sync.dma_start(qsw[32 * bb:32 * bb + 32, :],
                                          qpe_b[32 * sb:32 * sb + 32, :])
                    t1 = sbt.tile([P, SS], F32, tag="t1")
                    nc.vector.tensor_mul(t1[:], qpe_b[:], cos_t[:, sl])
                    t2 = sbt.tile([P, SS], F32, tag="t2")
                    nc.vector.tensor_mul(t2[:], qsw[:], sin_t[:, sl])
                    qrot = sbt.tile([P, SS], BF16, tag="qrot")
                    nc.vector.tensor_add(qrot[:], t1[:], t2[:])
                    nc.scalar.dma_start(qpe2[:, 0, sl], qrot[0:64, :])
                    nc.scalar.dma_start(qpe2[:, 1, sl], qrot[64:128, :])

                if DEBUG:
                    nc.sync.dma_start(dbg["dbg_kT"][:, :, :], kvasm[:, 0:2])
                    nc.sync.dma_start(dbg["dbg_qT"][:, :, :], qasm[:, :])

        # ================= attention + pipelined o_proj/post-LN =================
        with tc.tile_pool(name="sbe", bufs=6) as sbe, \
             tc.tile_pool(name="sbm", bufs=1) as sbm, \
             tc.tile_pool(name="scs", bufs=1) as scs, \
             tc.tile_pool(name="scr", bufs=1) as scr, \
             tc.tile_pool(name="pat", bufs=1, space="PSUM") as pat:
            wos = sbm.tile([P, 16, 2, P], BF16)
            nc.sync.dma_start(wos[:], wo_t.rearrange("a b p m -> p a b m"))

            def cproj_a(j):
                """o_proj chunk j + residual + sq-stats + AR4_j."""
                nsl = slice(j * 512, (j + 1) * 512)
                rhs = scr.tile([P, 16, 512], BF16, tag="rhs2", name="rhs2")
                nc.sync.dma_start(
                    rhs[:], ag2_out[j].rearrange("(kt p) s -> p kt s", p=P))
                resid = scs.tile([P, 2, 512], F32, tag="resid", name="resid")
                nc.sync.dma_start(
                    resid[:],
                    hT_r.rearrange("(mc p) s -> p mc s", p=P)[:, :, nsl])
                sqh = scs.tile([P, 2, 512], BF16, tag="sqh", name="sqh")
                for mc in range(2):
                    ps = pat.tile([P, 512], F32, tag="rb", bufs=1, name="omm")
                    for kt in range(16):
                        nc.tensor.matmul(ps[:], wos[:, kt, mc], rhs[:, kt],
                                         start=(kt == 0), stop=(kt == 15))
                    nc.vector.tensor_add(h2[:, mc, nsl], ps[:], resid[:, mc])
                    nc.vector.tensor_mul(sqh[:, mc], h2[:, mc, nsl],
                                         h2[:, mc, nsl])
                ps4 = pat.tile([1, 512], F32, tag="m4", bufs=1, name="m4")
                for mc in range(2):
                    nc.tensor.matmul(ps4[:], ones_r[:], sqh[:, mc],
                                     start=(mc == 0), stop=(mc == 1))
                msq4 = scs.tile([1, 512], F32, tag="msq4", name="msq4")
                nc.vector.tensor_copy(msq4[:], ps4[:])
                nc.sync.dma_start(ar4_in[j][:, :], msq4[:])
                nc.gpsimd.collective_compute(
                    "AllReduce", mybir.AluOpType.add, replica_groups=RG,
                    ins=[ar4_in[j]], outs=[ar4_out[j]])

            def cproj_b(j):
                """r4_j + yT_j + AG3_j."""
                nsl = slice(j * 512, (j + 1) * 512)
                msq4g = scs.tile([1, 512], F32, tag="m4g", name="msq4g")
                nc.sync.dma_start(msq4g[:], ar4_out[j][:, :])
                r4s = scs.tile([1, 512], F32, tag="r4s", name="r4s")
                nc.scalar.activation(r4s[:], msq4g[:], AF.Sqrt,
                                     scale=1.0 / H, bias=eps_t[:1])
                r4sr = scs.tile([1, 512], F32R, tag="r4sr", name="r4sr")
                nc.vector.tensor_copy(r4sr[:], r4s[:])
                r4bp = pat.tile([P, 512], F32, tag="rb", bufs=1, name="rb")
                nc.tensor.matmul(r4bp[:], ones_k1r[:], r4sr[:],
                                 start=True, stop=True)
                r4b = scs.tile([P, 512], F32, tag="r4b", name="r4b")
                nc.vector.reciprocal_approx_fast(r4b[:], r4bp[:])
                yT = scs.tile([P, 2, 512], BF16, tag="yT", name="yT")
                nc.vector.tensor_mul(
                    yT[:], h2[:, :, nsl],
                    r4b[:, None, :].to_broadcast([P, 2, 512]))
                nc.sync.dma_start(
                    ag3_in[j].rearrange("(mc p) s -> p mc s", p=P), yT[:])
                nc.gpsimd.collective_compute(
                    "AllGather", mybir.AluOpType.bypass, replica_groups=RG,
                    ins=[ag3_in[j]], outs=[ag3_out[j]])

            with nc.named_scope("stageB_attn"):
                mask_t = sbm.tile([P, 4, 512], BF16)
                nc.sync.dma_start(mask_t[:], dmask[:, :, :])
                for qc in range(4):
                    qsl = slice(qc * 512, (qc + 1) * 512)
                    nkt = 4 * qc + 4
                    o_ps = [pat.tile([P, 512], F32, tag="o", bufs=2,
                                     name=f"o{h}") for h in range(2)]
                    d_ps = [pat.tile([1, 512], F32, tag="d", bufs=2,
                                     name=f"d{h}") for h in range(2)]
                    for kt in range(nkt):
                        ksl = slice(kt * P, (kt + 1) * P)
                        j = kt - 4 * qc
                        for h in range(2):
                            sc_ps = pat.tile([P, 512], F32, tag="sc", bufs=2,
                                             name="scp")
                            nc.tensor.matmul(sc_ps[:], kvasm[:, h, ksl],
                                             qasm[:, h, qsl], start=True,
                                             stop=False)
                            nc.tensor.matmul(sc_ps[:], kpeT[:, ksl],
                                             qpe2[:, h, qsl], start=False,
                                             stop=True)
                            if j >= 0:
                                nc.vector.tensor_add(sc_ps[:], sc_ps[:],
                                                     mask_t[:, j])
                            es = sbe.tile([P, 512], BF16, tag="es", bufs=5)
                            nc.scalar.activation(es[:], sc_ps[:], AF.Exp)
                            nc.tensor.matmul(o_ps[h][:], v_tok[:, h, kt], es[:],
                                             start=(kt == 0), stop=(kt == nkt - 1))
                            nc.tensor.matmul(d_ps[h][:], ones_r[:], es[:],
                                             start=(kt == 0), stop=(kt == nkt - 1))
                    for h in range(2):
                        ds = sbe.tile([1, 512], F32R, tag="ds", bufs=2)
                        nc.vector.tensor_copy(ds[:], d_ps[h][:])
                        rb_ps = pat.tile([P, 512], F32, tag="rb", bufs=1,
                                         name="rbo")
                        nc.tensor.matmul(rb_ps[:], ones_k1r[:], ds[:],
                                         start=True, stop=True)
                        recb = sbe.tile([P, 512], F32, tag="recb", bufs=2)
                        nc.vector.reciprocal_approx_fast(recb[:], rb_ps[:])
                        nc.vector.tensor_mul(oT[:, h, qsl], o_ps[h][:], recb[:])
                    nc.sync.dma_start(
                        ag2_in[qc].rearrange("(mc p) s -> p mc s", p=P),
                        oT[:, :, qsl])
                    nc.gpsimd.collective_compute(
                        "AllGather", mybir.AluOpType.bypass, replica_groups=RG,
                        ins=[ag2_in[qc]], outs=[ag2_out[qc]])
                    if qc >= 1:
                        cproj_a(qc - 1)
                    if qc >= 2:
                        cproj_b(qc - 2)
                cproj_a(3)
                cproj_b(2)
                cproj_b(3)
                if DEBUG:
                    nc.sync.dma_start(dbg["dbg_oT"][:, :, :], oT[:])
                    nc.sync.dma_start(dbg["dbg_h2"][:, :, :], h2[:])

        bpool_cm.__exit__(None, None, None)

        # ================= Stage D: merged MLP pipeline =================
        with tc.tile_pool(name="sd", bufs=1) as sd, \
             tc.tile_pool(name="sdr", bufs=2) as sdr, \
             tc.tile_pool(name="sde", bufs=4) as sde, \
             tc.tile_pool(name="sdd", bufs=6) as sdd, \
             tc.tile_pool(name="pdg", bufs=2, space="PSUM") as pdg:
            with nc.named_scope("stageD"):
                wds = sd.tile([P, 8, 16, P], BF16)
                for mc in range(16):
                    nc.gpsimd.dma_start(wds[:, :, mc, :],
                                        wd_t[:, mc].rearrange("a p m -> p a m"))
                act = sd.tile([P, 8, 512], BF16, name="act")  # per-ncol activations

                # rs chunk layout: (ncol, col offset within ncol, width, rs idx)
                CH = {0: [(0, 512, 0)], 1: [(0, 512, 1)], 2: [(0, 512, 2)],
                      3: [(0, 256, 3), (256, 256, 4)]}
                for ncol in range(4):
                    nsl = slice(ncol * 512, (ncol + 1) * 512)
                    rhs = sdr.tile([P, 16, 512], BF16, tag="rhs3")
                    nc.sync.dma_start(
                        rhs[:], ag3_out[ncol].rearrange("(kt p) s -> p kt s", p=P))
                    for m in range(8):
                        gp = pdg.tile([P, 512], F32, tag="g", name="gps")
                        up = pdg.tile([P, 512], F32, tag="u", name="ups")
                        for kt in range(16):
                            nc.tensor.matmul(gp[:], wgs[:, kt, m], rhs[:, kt],
                                             start=(kt == 0), stop=(kt == 15))
                            nc.tensor.matmul(up[:], wus[:, kt, m], rhs[:, kt],
                                             start=(kt == 0), stop=(kt == 15))
                        gsil = sde.tile([P, 512], BF16, tag="gsil")
                        nc.scalar.activation(gsil[:], gp[:], AF.Silu)
                        nc.vector.tensor_mul(act[:, m], gsil[:], up[:])
                    # down projection for this token-column block
                    for (c0, cw, jr) in CH[ncol]:
                        csl = slice(c0, c0 + cw)
                        gsl = slice(ncol * 512 + c0, ncol * 512 + c0 + cw)
                        for mc in range(16):
                            ps = pdg.tile([P, 512], F32, tag="dmm",
                                          name="dmmps")[:, :cw]
                            for kt in range(8):
                                nc.tensor.matmul(ps[:], wds[:, kt, mc],
                                                 act[:, kt, csl],
                                                 start=(kt == 0), stop=(kt == 7))
                            dn = sdd.tile([P, 512], BF16, tag="dn",
                                          name="dntile")[:, :cw]
                            if mc % 2 == 0:
                                nc.vector.tensor_copy(dn[:], ps[:])
                            else:
                                nc.scalar.activation(dn[:], ps[:], AF.Copy)
                            nc.sync.dma_start(rs_in[jr][mc * P:(mc + 1) * P, :],
                                              dn[:])
                        nc.gpsimd.collective_compute(
                            "ReduceScatter", mybir.AluOpType.add,
                            replica_groups=RG,
                            ins=[rs_in[jr]], outs=[rs_out[jr]])
                        fin = sdd.tile([P, 2, 512], BF16, tag="fin", bufs=2,
                                       name="fintile")[:, :, :cw]
                        nc.sync.dma_start(
                            fin[:],
                            rs_out[jr].rearrange("(mc p) s -> p mc s", p=P))
                        fino = sdd.tile([P, 2, 512], F32, tag="fino", bufs=2,
                                        name="finotile")[:, :, :cw]
                        nc.vector.tensor_add(fino[:], fin[:], h2[:, :, gsl])
                        nc.sync.dma_start(
                            outT.rearrange("(mc p) s -> p mc s", p=P)[:, :, gsl],
                            fino[:])

    nc.compile()
    _CACHE["nc"] = nc
    return nc


def _host_prep(inputs):
    import ml_dtypes
    bf16 = ml_dtypes.bfloat16
    inp = {k: np.asarray(v) for k, v in inputs.items()}
    hidden = inp["hidden_states"].reshape(S, H).astype(np.float32)
    pos = inp["position_ids"].reshape(S).astype(np.int64)
    cosT = inp["cos"][pos].T.astype(np.float32)
    sinT = inp["sin"][pos].T.astype(np.float32)
    wq_a = (inp["wq_a"] * inp["in_ln"][:, None]).astype(np.float32)
    wkv_a = (inp["wkv_a"] * inp["in_ln"][:, None]).astype(np.float32)
    wq_b = (inp["wq_b"] * inp["q_a_ln"][:, None]).astype(np.float32)
    wkv_b = (inp["wkv_b"] * inp["kv_a_ln"][:, None]).astype(np.float32)
    wg = (inp["w_gate"] * inp["post_ln"][:, None]).astype(np.float32)
    wu = (inp["w_up"] * inp["post_ln"][:, None]).astype(np.float32)
    wd = inp["w_down"].astype(np.float32)
    wo = inp["wo"].astype(np.float32)

    de = np.empty(ROPE, np.int64)
    de[:32] = np.arange(32) * 2
    de[32:] = np.arange(32) * 2 + 1
    wkv_a = np.concatenate([wkv_a[:, :KVLR], wkv_a[:, KVLR:][:, de]], axis=1)
    wq_b = wq_b.reshape(QLR, NH, QHD)
    wkv_b = wkv_b.reshape(KVLR, NH, NOPE + VHD)

    hT = hidden.T.copy()
    sin_sg = np.concatenate([-sinT[:32], sinT[32:]], axis=0)    # signed for swap trick
    cossin = np.concatenate([cosT, cosT, sin_sg, sin_sg], axis=0)  # (256, S)
    ki = np.arange(P)[:, None]
    qi = np.arange(512)[None, :]
    dmask = np.stack([np.where(qi >= j * P + ki, 0.0, -1e30).astype(np.float32)
                      for j in range(4)], axis=1).astype(bf16)  # (128, 4, 512)

    wq_a_t = _tile_w(wq_a).astype(bf16)
    wkv_a_t = _tile_w(wkv_a).astype(bf16)

    in_maps = []
    for c in range(NC):
        h0, h1 = 2 * c, 2 * c + 1
        qb = np.concatenate([
            wq_b[:, h0, :NOPE], wq_b[:, h1, :NOPE],
            wq_b[:, h0, NOPE:][:, de], wq_b[:, h1, NOPE:][:, de]], axis=1) * SCALE
        kb = np.concatenate([
            wkv_b[:, h0, :NOPE], wkv_b[:, h1, :NOPE],
            wkv_b[:, h0, NOPE:], wkv_b[:, h1, NOPE:]], axis=1)
        ssl = slice(c * SS, (c + 1) * SS)
        cs_sh = np.concatenate([cosT[:, ssl], sin_sg[:, ssl]], axis=0)
        in_maps.append({
            "hT_s": np.ascontiguousarray(hT[:, ssl]),
            "hT_r": np.ascontiguousarray(hT[ssl, :]),
            "wq_a_t": wq_a_t,
            "wkv_a_t": wkv_a_t,
            "wq_b_t": _tile_w(qb.astype(np.float32)).astype(bf16),
            "wkv_b_t": _tile_w(kb.astype(np.float32)).astype(bf16),
            "wo_t": _tile_w(np.ascontiguousarray(wo[:, ssl])).astype(bf16),
            "wg_t": _tile_w(wg[:, c * FFS:(c + 1) * FFS]).astype(bf16),
            "wu_t": _tile_w(wu[:, c * FFS:(c + 1) * FFS]).astype(bf16),
            "wd_t": _tile_w(wd[c * FFS:(c + 1) * FFS, :]).astype(bf16),
            "cossin": cossin.astype(bf16),
            "cs_sh": np.ascontiguousarray(cs_sh),
            "dmask": dmask,
        })
    return in_maps


_LAST_RESULT = {}


def kernel(**inputs) -> np.ndarray:
    from concourse.bass_utils import run_bass_kernel_spmd
    nc = _build()
    in_maps = _host_prep(inputs)
    kwargs = {}
    if TRACE:
        import sys, types
        if "antenv.axon_hooks" not in sys.modules:
            try:
                from trn_agent_boot.trn_boot import _ntff_profile_via_ctypes
                mod = types.ModuleType("antenv.axon_hooks")
                _hook = _ntff_profile_via_ctypes('/opt/axon/libaxon_pjrt.so')
                mod.get_axon_ntff_profile_hook = lambda: _hook
                mod.set_axon_ntff_profile_hook = lambda h: None
                sys.modules["antenv.axon_hooks"] = mod
                import antenv
                antenv.axon_hooks = mod
            except Exception:
                pass
        kwargs["trace"] = True
    res = run_bass_kernel_spmd(nc, in_maps, list(range(NC)), **kwargs)
    _LAST_RESULT["res"] = res
    outT = np.concatenate([res.results[c]["outT"] for c in range(NC)], axis=0)
    return np.ascontiguousarray(outT.T)[None].astype(np.float32)
